# revision 4
# baseline (speedup 1.0000x reference)
"""Trainium2 Bass kernel for nn_AMK_Block (sparse_attention), 8 NeuronCores.

Sharding: core c => (batch b = c//2, seq half h = c%2), T=2048 tokens/core.
Collectives: q_pool AllReduce (8 cores), Omega AllGather (8, hyper GEMM
row-sharded), C/phi_k_sum pair AllReduce, conv-halo pair AllGather.
Heavy GEMMs in bf16 (fp32 accumulate); norms/elementwise mostly fp32.
"""
import os
import sys
import types
import numpy as np

import concourse.bass as bass
import concourse.mybir as mybir
import concourse.tile as tile
from concourse import bacc
from concourse.bass_utils import run_bass_kernel_spmd
from concourse.masks import make_identity

F32 = mybir.dt.float32
BF16 = mybir.dt.bfloat16
AF = mybir.ActivationFunctionType
ALU = mybir.AluOpType
AX = mybir.AxisListType

NCORES = 8
B, N, DM, DS = 4, 4096, 1024, 64
INNER = 4 * DM
T = N // 2               # tokens per core
NT = T // 128            # 16 tok tiles
ND = DM // 128           # 8 d tiles
NIH = INNER // 128 // 2  # 16 i-tiles per inner half
RS = DM * DS // NCORES   # 8192 hyper rows per core
TPAD = T + 4             # H free dim with halo pad (cols 1..2050 used)

_cache = {}


def _install_ntff_shim():
    if "antenv.axon_hooks" in sys.modules:
        return
    try:
        from trn_agent_boot.trn_boot import _ntff_profile_via_ctypes
        hook = _ntff_profile_via_ctypes("/opt/axon/libaxon_pjrt.so")
    except Exception:
        hook = None
    m = types.ModuleType("antenv.axon_hooks")
    m.get_axon_ntff_profile_hook = lambda: hook
    m.set_axon_ntff_profile_hook = lambda h: None
    sys.modules["antenv.axon_hooks"] = m


def build(dt_safe: float, lam_safe: float):
    nc = bacc.Bacc(None, target_bir_lowering=False, debug=False)

    q_in = nc.dram_tensor("q_in", [T, DM], F32, kind="ExternalInput")
    x_in = nc.dram_tensor("x_in", [T, DM], F32, kind="ExternalInput")
    cosn = nc.dram_tensor("cosn", [T, DM], F32, kind="ExternalInput")
    sinn = nc.dram_tensor("sinn", [T, DM], F32, kind="ExternalInput")
    hqk = nc.dram_tensor("hqk", [2, DM, RS], F32, kind="ExternalInput")
    wupT = nc.dram_tensor("wupT", [DM, 2 * INNER], F32, kind="ExternalInput")
    wdownT = nc.dram_tensor("wdownT", [INNER, DM], F32, kind="ExternalInput")
    mprojT = nc.dram_tensor("mprojT", [DM, DM], F32, kind="ExternalInput")
    nrm_bc = nc.dram_tensor("nrm_bc", [5, 128, DM], F32, kind="ExternalInput")
    bias_q = nc.dram_tensor("bias_q", [DS, 2], F32, kind="ExternalInput")
    bias_k_bc = nc.dram_tensor("bias_k_bc", [128, DS], F32, kind="ExternalInput")
    dwk = nc.dram_tensor("dwk", [128, 2 * NIH, 3], F32, kind="ExternalInput")
    bsel = nc.dram_tensor("bsel", [128, B], F32, kind="ExternalInput")
    lrsel = nc.dram_tensor("lrsel", [128, 4], F32, kind="ExternalInput")
    out_ext = nc.dram_tensor("out", [T, DM], F32, kind="ExternalOutput")

    GRP_ALL = [list(range(NCORES))]
    GRP_PAIR = [[2 * i, 2 * i + 1] for i in range(B)]

    def ln_tile(pool, stats, xf, g_bc, b_bc, eps_ap):
        """LayerNorm over free dim of xf [128, DM] f32; returns f32 tile."""
        smu = stats.tile([128, 1], F32, tag="smu", bufs=3, name="smu")
        nc.vector.tensor_reduce(smu[:], xf[:], AX.X, ALU.add)
        negmu = stats.tile([128, 1], F32, tag="negmu", bufs=3, name="negmu")
        nc.vector.tensor_scalar(negmu[:], smu[:], -1.0 / DM, None, ALU.mult)
        xc = pool.tile([128, DM], F32, tag="ln_xc", bufs=2, name="ln_xc")
        nc.vector.tensor_scalar(xc[:], xf[:], negmu[:], None, ALU.add)
        sq = pool.tile([128, DM], F32, tag="ln_sq", bufs=2, name="ln_sq")
        vs = stats.tile([128, 1], F32, tag="vs", bufs=3, name="vs")
        nc.scalar.activation(sq[:], xc[:], AF.Square, accum_out=vs[:])
        std = stats.tile([128, 1], F32, tag="std", bufs=3, name="std")
        nc.scalar.activation(std[:], vs[:], AF.Sqrt, bias=eps_ap, scale=1.0 / DM)
        rs = stats.tile([128, 1], F32, tag="rs", bufs=3, name="rs")
        nc.vector.reciprocal(rs[:], std[:])
        xn = pool.tile([128, DM], F32, tag="ln_xn", bufs=2, name="ln_xn")
        nc.vector.tensor_scalar(xn[:], xc[:], rs[:], None, ALU.mult)
        nc.vector.tensor_tensor(xc[:], xn[:], g_bc[:], ALU.mult)
        lnout = pool.tile([128, DM], F32, tag="ln_out", bufs=2, name="ln_out")
        nc.vector.tensor_tensor(lnout[:], xc[:], b_bc[:], ALU.add)
        return lnout

    with tile.TileContext(nc) as tc:
        with tc.tile_pool(name="dram", bufs=1, space="DRAM") as dram, \
             tc.tile_pool(name="const", bufs=1) as constp, \
             tc.tile_pool(name="keep", bufs=1) as keep, \
             tc.tile_pool(name="stats", bufs=1) as stats:

            # ---------------- DRAM scratch ----------------
            def dt_(shape, dtype, nm, shared=False):
                return dram.tile(shape, dtype, tag=nm, name=nm,
                                 addr_space="Shared" if shared else "Local")
            qp_in = dt_([B, DM], F32, "qp_in")
            qp_out = dt_([B, DM], F32, "qp_out", True)
            om_in = dt_([2, B, RS], F32, "om_in")
            om_out = dt_([NCORES, 2, B, RS], F32, "om_out", True)
            c_in = dt_([DS, DM + 1], F32, "c_in")
            c_out = dt_([DS, DM + 1], F32, "c_out")
            qn1_sp = dt_([T, DM], BF16, "qn1_sp")
            base_sp = dt_([T, DM], BF16, "base_sp")
            qint_sp = dt_([T, DM], BF16, "qint_sp")
            halo_in = [dt_([128, 2 * NIH], BF16, f"halo_in{k}") for k in range(2)]
            halo_out = [dt_([2, 128, 2 * NIH], BF16, f"halo_out{k}")
                        for k in range(2)]

            # ---------------- constants ----------------
            ones_b = constp.tile([128, 1], BF16, tag="ones_b", name="ones_b")
            nc.vector.memset(ones_b[:], 1.0)
            eps_c = constp.tile([128, 1], F32, tag="eps_c", name="eps_c")
            nc.vector.memset(eps_c[:], 1e-5)
            ident = constp.tile([128, 128], F32, tag="ident", name="ident")
            make_identity(nc, ident)
            bq = constp.tile([DS, 2], F32, tag="bq", name="bq")
            nc.sync.dma_start(bq[:], bias_q[:])
            bk_bc = constp.tile([128, DS], F32, tag="bk_bc", name="bk_bc")
            nc.sync.dma_start(bk_bc[:], bias_k_bc[:])
            dwk_sb = constp.tile([128, 2 * NIH, 3], F32, tag="dwk", name="dwk_sb")
            nc.sync.dma_start(dwk_sb[:], dwk[:])
            bsel_sb = constp.tile([128, B], F32, tag="bsel", name="bsel_sb")
            nc.sync.dma_start(bsel_sb[:], bsel[:])
            lrsel_sb = constp.tile([128, 4], F32, tag="lrsel", name="lrsel_sb")
            nc.sync.dma_start(lrsel_sb[:], lrsel[:])
            n2g_bc = constp.tile([128, DM], F32, tag="n2g", name="n2g_bc")
            nc.sync.dma_start(n2g_bc[:], nrm_bc[2])
            n2b_bc = constp.tile([128, DM], F32, tag="n2b", name="n2b_bc")
            nc.sync.dma_start(n2b_bc[:], nrm_bc[3])

            # long-lived across phases
            qn2T = keep.tile([128, ND, T], BF16, tag="qn2T", name="qn2T")
            rN = keep.tile([128, NT], F32, tag="rN", name="rN")

            with tc.tile_pool(name="transp", bufs=1) as transp:
                qrotT = transp.tile([128, ND, T], BF16, tag="tbig", name="qrotT")

                # ============ phase 1 ============
                with tc.tile_pool(name="w1", bufs=1) as w1, \
                     tc.tile_pool(name="ps1", bufs=1, space="PSUM") as ps1:
                    n1g_bc = w1.tile([128, DM], F32, tag="n1g", name="n1g_bc")
                    nc.sync.dma_start(n1g_bc[:], nrm_bc[0])
                    n1b_bc = w1.tile([128, DM], F32, tag="n1b", name="n1b_bc")
                    nc.sync.dma_start(n1b_bc[:], nrm_bc[1])
                    bb_bc = w1.tile([128, DM], F32, tag="bb", name="bb_bc")
                    nc.sync.dma_start(bb_bc[:], nrm_bc[4])
                    psqp = [ps1.tile([1, 512], F32, tag="qp", bufs=2, name=f"psqp{k}")
                            for k in range(2)]

                    for i in range(NT):
                        r = slice(i * 128, (i + 1) * 128)
                        qt = w1.tile([128, DM], F32, tag="qt", bufs=2, name="qt")
                        nc.sync.dma_start(qt[:], q_in[r, :])
                        xt = w1.tile([128, DM], F32, tag="xt", bufs=2, name="xt")
                        nc.sync.dma_start(xt[:], x_in[r, :])
                        ct = w1.tile([128, DM], F32, tag="ct", bufs=2, name="ct")
                        nc.sync.dma_start(ct[:], cosn[r, :])
                        st = w1.tile([128, DM], F32, tag="st", bufs=2, name="st")
                        nc.sync.dma_start(st[:], sinn[r, :])

                        qn1f = ln_tile(w1, stats, qt, n1g_bc, n1b_bc, eps_c[:])
                        qn1b = w1.tile([128, DM], BF16, tag="qn1b", bufs=2,
                                       name="qn1b")
                        nc.vector.tensor_copy(qn1b[:], qn1f[:])
                        nc.sync.dma_start(qn1_sp[r, :], qn1b[:])

                        for hf in range(2):
                            cs = slice(hf * 512, (hf + 1) * 512)
                            nc.tensor.matmul(psqp[hf][:], ones_b[:], qn1b[:, cs],
                                             start=(i == 0), stop=(i == NT - 1))

                        t1 = w1.tile([128, DM], F32, tag="t1", bufs=2, name="t1")
                        nc.vector.tensor_tensor(t1[:], qn1f[:], ct[:], ALU.mult)
                        u1 = w1.tile([128, 512], F32, tag="u1", bufs=2, name="u1")
                        nc.vector.tensor_tensor(u1[:], qn1f[:, 512:], st[:, :512],
                                                ALU.mult)
                        qr = w1.tile([128, DM], BF16, tag="qr", bufs=2, name="qr")
                        nc.vector.tensor_tensor(qr[:, :512], t1[:, :512], u1[:],
                                                ALU.subtract)
                        u2 = w1.tile([128, 512], F32, tag="u2", bufs=2, name="u2")
                        nc.vector.tensor_tensor(u2[:], qn1f[:, :512], st[:, 512:],
                                                ALU.mult)
                        nc.vector.tensor_tensor(qr[:, 512:], t1[:, 512:], u2[:],
                                                ALU.add)
                        for dj in range(ND):
                            nc.sync.dma_start(qrotT[:, dj, r],
                                              qr[:, dj * 128:(dj + 1) * 128],
                                              transpose=True)

                        bt = w1.tile([128, DM], F32, tag="bt", bufs=2, name="bt")
                        nc.vector.tensor_scalar(bt[:], qt[:], 1.0 - lam_safe, None,
                                                ALU.mult)
                        nc.vector.scalar_tensor_tensor(bt[:], xt[:], lam_safe, bt[:],
                                                       ALU.mult, ALU.add)
                        baseb = w1.tile([128, DM], BF16, tag="baseb", bufs=2,
                                        name="baseb")
                        nc.vector.tensor_tensor(baseb[:], bt[:], bb_bc[:], ALU.add)
                        nc.sync.dma_start(base_sp[r, :], baseb[:])

                    # ---- q_pool allreduce ----
                    qp_stage = w1.tile([1, DM], F32, tag="qp_stage", name="qp_stage")
                    z4 = w1.tile([B, DM], F32, tag="z4", name="z4")
                    for hf in range(2):
                        cs = slice(hf * 512, (hf + 1) * 512)
                        nc.scalar.activation(qp_stage[:, cs], psqp[hf][:], AF.Copy,
                                             scale=1.0 / N)
                        ps4 = ps1.tile([B, 512], F32, tag="b4", bufs=2, name="ps4")
                        nc.tensor.matmul(ps4[:], bsel_sb[0:1, :], qp_stage[:, cs],
                                         start=True, stop=True)
                        nc.scalar.activation(z4[:, cs], ps4[:], AF.Copy)
                    nc.gpsimd.dma_start(qp_in[:], z4[:])
                    nc.gpsimd.collective_compute(
                        "AllReduce", ALU.add, replica_groups=GRP_ALL,
                        ins=[qp_in.opt()], outs=[qp_out.opt()])

                # ============ phase 2: hyper GEMM + Omega allgather ============
                with tc.tile_pool(name="w2", bufs=1) as w2, \
                     tc.tile_pool(name="ps2", bufs=1, space="PSUM") as ps2:
                    qp4 = w2.tile([B, DM], F32, tag="qp4", name="qp4")
                    nc.gpsimd.dma_start(qp4[:], qp_out[:])
                    qpT = w2.tile([128, ND, B], BF16, tag="qpT", name="qpT")
                    for dj in range(ND):
                        pst = ps2.tile([128, B], F32, tag="tp", bufs=2, name="pst")
                        nc.tensor.transpose(pst[:], qp4[:, dj * 128:(dj + 1) * 128],
                                            ident[0:B, 0:B])
                        nc.scalar.activation(qpT[:, dj, :], pst[:], AF.Copy)

                    for mat in range(2):
                        for rc in range(RS // 512):
                            rcs = slice(rc * 512, (rc + 1) * 512)
                            hk = w2.tile([128, ND, 512], BF16, tag="hk", bufs=3,
                                         name="hk")
                            nc.gpsimd.dma_start(
                                hk[:],
                                hqk[mat, :, rcs].rearrange("(dj p) r -> p dj r",
                                                           p=128))
                            pso = ps2.tile([B, 512], F32, tag="b4", bufs=2,
                                           name="pso")
                            for dj in range(ND):
                                nc.tensor.matmul(pso[:], qpT[:, dj, :], hk[:, dj, :],
                                                 start=(dj == 0),
                                                 stop=(dj == ND - 1))
                            st512 = w2.tile([B, 512], F32, tag="st512", bufs=3,
                                            name="st512")
                            nc.scalar.activation(st512[:], pso[:], AF.Copy)
                            nc.sync.dma_start(om_in[mat, :, rcs], st512[:])
                    nc.gpsimd.collective_compute(
                        "AllGather", ALU.bypass, replica_groups=GRP_ALL,
                        ins=[om_in.opt()], outs=[om_out.opt()])

                # ============ phases 3-5 ============
                with tc.tile_pool(name="w3", bufs=1) as w3:
                    om_sb = w3.tile([128, 2, ND, DS], BF16, tag="om_sb",
                                    name="om_sb")
                    for mat in range(2):
                        for dj in range(ND):
                            for b in range(B):
                                obt = w3.tile([128, DS], BF16, tag="obt", bufs=4,
                                              name="obt")
                                nc.gpsimd.dma_start(
                                    obt[:],
                                    om_out[dj, mat, b].rearrange("(p e) -> p e",
                                                                 p=128))
                                if b == 0:
                                    nc.vector.tensor_scalar(
                                        om_sb[:, mat, dj, :], obt[:],
                                        bsel_sb[:, 0:1], None, ALU.mult)
                                else:
                                    nc.vector.scalar_tensor_tensor(
                                        om_sb[:, mat, dj, :], obt[:],
                                        bsel_sb[:, b:b + 1], om_sb[:, mat, dj, :],
                                        ALU.mult, ALU.add)

                    phiK = w3.tile([128, NT, DS], BF16, tag="phiK", name="phiK")
                    phiQT = w3.tile([DS, NT, 128], BF16, tag="phiQT", name="phiQT")
                    with tc.tile_pool(name="ps3", bufs=1, space="PSUM") as ps3:
                        for i in range(NT):
                            r = slice(i * 128, (i + 1) * 128)
                            pk = ps3.tile([128, DS], F32, tag="phi", bufs=2,
                                          name="pk")
                            for dj in range(ND):
                                nc.tensor.matmul(pk[:], qrotT[:, dj, r],
                                                 om_sb[:, 1, dj, :],
                                                 start=(dj == 0),
                                                 stop=(dj == ND - 1))
                            zf = w3.tile([128, DS], F32, tag="zf", bufs=2, name="zf")
                            nc.vector.tensor_tensor(zf[:], pk[:], bk_bc[:], ALU.add)
                            rf = w3.tile([128, DS], F32, tag="rf", bufs=2, name="rf")
                            nc.scalar.activation(rf[:], zf[:], AF.Relu)
                            ef = w3.tile([128, DS], F32, tag="ef", bufs=2, name="ef")
                            nc.vector.tensor_tensor(ef[:], zf[:], rf[:],
                                                    ALU.subtract)
                            nc.scalar.activation(ef[:], ef[:], AF.Exp)
                            nc.vector.tensor_tensor(phiK[:, i, :], rf[:], ef[:],
                                                    ALU.add)

                            pq = ps3.tile([DS, 128], F32, tag="phiq", bufs=2,
                                          name="pq")
                            for dj in range(ND):
                                nc.tensor.matmul(pq[:], om_sb[:, 0, dj, :],
                                                 qrotT[:, dj, r],
                                                 start=(dj == 0),
                                                 stop=(dj == ND - 1))
                            t1q = w3.tile([DS, 128], F32, tag="t1q", bufs=2,
                                          name="t1q")
                            nc.scalar.activation(t1q[:], pq[:], AF.Relu,
                                                 bias=bq[:, 0:1])
                            t2q = w3.tile([DS, 128], F32, tag="t2q", bufs=2,
                                          name="t2q")
                            nc.scalar.activation(t2q[:], pq[:], AF.Relu,
                                                 bias=bq[:, 1:2], scale=-1.0)
                            nc.scalar.activation(t2q[:], t2q[:], AF.Exp, scale=-1.0)
                            nc.vector.tensor_tensor(phiQT[:, i, :], t1q[:], t2q[:],
                                                    ALU.add)

                        # ---- C GEMM + phi_k_sum + pair allreduce ----
                        psc = [ps3.tile([DS, 512], F32, tag="c", bufs=2,
                                        name=f"psc{k}") for k in range(2)]
                        psk = ps3.tile([DS, 1], F32, tag="pks", bufs=1, name="psk")
                        for i in range(NT):
                            qn1t = w3.tile([128, DM], BF16, tag="qn1t", bufs=3,
                                           name="qn1t")
                            nc.sync.dma_start(qn1t[:],
                                              qn1_sp[i * 128:(i + 1) * 128, :])
                            for nb in range(2):
                                nc.tensor.matmul(psc[nb][:], phiK[:, i, :],
                                                 qn1t[:, nb * 512:(nb + 1) * 512],
                                                 start=(i == 0), stop=(i == NT - 1))
                            nc.tensor.matmul(psk[:], phiK[:, i, :], ones_b[:],
                                             start=(i == 0), stop=(i == NT - 1))
                        c_stage = w3.tile([DS, DM + 1], F32, tag="c_stage",
                                          name="c_stage")
                        for nb in range(2):
                            nc.scalar.activation(
                                c_stage[:, nb * 512:(nb + 1) * 512], psc[nb][:],
                                AF.Copy)
                        nc.scalar.activation(c_stage[:, DM:DM + 1], psk[:], AF.Copy)
                        nc.gpsimd.dma_start(c_in[:], c_stage[:])
                        nc.gpsimd.collective_compute(
                            "AllReduce", ALU.add, replica_groups=GRP_PAIR,
                            ins=[c_in.opt()], outs=[c_out.opt()])

                    c_b = w3.tile([DS, DM], BF16, tag="c_b", name="c_b")
                    nc.gpsimd.dma_start(c_b[:], c_out[:, 0:DM])
                    pks_b = w3.tile([DS, 1], BF16, tag="pks_b", name="pks_b")
                    nc.gpsimd.dma_start(pks_b[:], c_out[:, DM:DM + 1])

                    mprojb = w3.tile([128, ND, DM], BF16, tag="mprojb",
                                     name="mprojb")
                    nc.gpsimd.dma_start(
                        mprojb[:],
                        mprojT[:, :].rearrange("(dj p) n -> p dj n", p=128))

                    # ---- phase 5 ----
                    mT = transp.tile([128, ND, T], BF16, tag="tbig", name="mT")
                    with tc.tile_pool(name="ps5", bufs=1, space="PSUM") as ps5:
                        for i in range(NT):
                            r = slice(i * 128, (i + 1) * 128)
                            pd = ps5.tile([128, 1], F32, tag="d1", bufs=2, name="pd")
                            nc.tensor.matmul(pd[:], phiQT[:, i, :], pks_b[:],
                                             start=True, stop=True)
                            ab = stats.tile([128, 1], F32, tag="ab", bufs=2,
                                            name="ab")
                            nc.scalar.activation(ab[:], pd[:], AF.Abs)
                            nc.vector.tensor_scalar(ab[:], ab[:], 1.0, None, ALU.add)
                            nc.vector.reciprocal(rN[:, i:i + 1], ab[:])

                            qn1t = w3.tile([128, DM], BF16, tag="qn1t", bufs=3,
                                           name="qn1t")
                            nc.sync.dma_start(qn1t[:], qn1_sp[r, :])
                            mb = w3.tile([128, DM], BF16, tag="mb", bufs=2,
                                         name="mb")
                            for nb in range(2):
                                cs = slice(nb * 512, (nb + 1) * 512)
                                pa = ps5.tile([128, 512], F32, tag="mm512", bufs=4,
                                              name="pa")
                                nc.tensor.matmul(pa[:], phiQT[:, i, :], c_b[:, cs],
                                                 start=True, stop=True)
                                tb = w3.tile([128, 512], BF16, tag="tb", bufs=2,
                                             name="tb")
                                nc.vector.tensor_scalar(tb[:], pa[:], rN[:, i:i + 1],
                                                        None, ALU.mult)
                                nc.vector.tensor_tensor(mb[:, cs], tb[:],
                                                        qn1t[:, cs], ALU.subtract)
                            for dj in range(ND):
                                nc.sync.dma_start(mT[:, dj, r],
                                                  mb[:, dj * 128:(dj + 1) * 128],
                                                  transpose=True)

                            baset = w3.tile([128, DM], BF16, tag="baset", bufs=2,
                                            name="baset")
                            nc.sync.dma_start(baset[:], base_sp[r, :])
                            qint = w3.tile([128, DM], F32, tag="qint", bufs=2,
                                           name="qint")
                            for nb in range(2):
                                cs = slice(nb * 512, (nb + 1) * 512)
                                pm = ps5.tile([128, 512], F32, tag="mm512", bufs=4,
                                              name="pm")
                                for dj in range(ND):
                                    nc.tensor.matmul(pm[:], mT[:, dj, r],
                                                     mprojb[:, dj, cs],
                                                     start=(dj == 0),
                                                     stop=(dj == ND - 1))
                                tb2 = w3.tile([128, 512], BF16, tag="tb2", bufs=2,
                                              name="tb2")
                                nc.vector.tensor_scalar(tb2[:], pm[:], dt_safe,
                                                        None, ALU.mult)
                                nc.vector.tensor_tensor(qint[:, cs], tb2[:],
                                                        baset[:, cs], ALU.add)
                            qint_b = w3.tile([128, DM], BF16, tag="qint_b", bufs=2,
                                             name="qint_b")
                            nc.vector.tensor_copy(qint_b[:], qint[:])
                            nc.sync.dma_start(qint_sp[r, :], qint_b[:])
                            qn2f = ln_tile(w3, stats, qint, n2g_bc, n2b_bc, eps_c[:])
                            qn2b = w3.tile([128, DM], BF16, tag="qn2b", bufs=2,
                                           name="qn2b")
                            nc.vector.tensor_copy(qn2b[:], qn2f[:])
                            for dj in range(ND):
                                nc.sync.dma_start(qn2T[:, dj, r],
                                                  qn2b[:, dj * 128:(dj + 1) * 128],
                                                  transpose=True)

            # ============ phases 6-7: FFN ============
            with tc.tile_pool(name="ffn", bufs=1) as ffn, \
                 tc.tile_pool(name="ws", bufs=1) as ws, \
                 tc.tile_pool(name="ps7", bufs=1, space="PSUM") as ps7:
                acc = ffn.tile([128, NT, DM], BF16, tag="acc", name="acc")
                H = ffn.tile([128, NIH, TPAD], BF16, tag="H", name="H")
                for half in range(2):
                    ibase = half * NIH
                    for mi in range(NIH):
                        g0 = half * 2048 + mi * 128
                        u0 = INNER + g0
                        wu_g = ws.tile([128, ND, 128], BF16, tag="wu_g", bufs=3,
                                       name="wu_g")
                        nc.gpsimd.dma_start(
                            wu_g[:],
                            wupT[:, g0:g0 + 128].rearrange("(dj p) f -> p dj f",
                                                           p=128))
                        wu_u = ws.tile([128, ND, 128], BF16, tag="wu_u", bufs=3,
                                       name="wu_u")
                        nc.gpsimd.dma_start(
                            wu_u[:],
                            wupT[:, u0:u0 + 128].rearrange("(dj p) f -> p dj f",
                                                           p=128))
                        for nb in range(4):
                            cs = slice(nb * 512, (nb + 1) * 512)
                            hs = slice(2 + nb * 512, 2 + (nb + 1) * 512)
                            pg = ps7.tile([128, 512], F32, tag="gu", bufs=4,
                                          name="pg")
                            for dj in range(ND):
                                nc.tensor.matmul(pg[:], wu_g[:, dj, :],
                                                 qn2T[:, dj, cs],
                                                 start=(dj == 0),
                                                 stop=(dj == ND - 1))
                            pu = ps7.tile([128, 512], F32, tag="gu", bufs=4,
                                          name="pu")
                            for dj in range(ND):
                                nc.tensor.matmul(pu[:], wu_u[:, dj, :],
                                                 qn2T[:, dj, cs],
                                                 start=(dj == 0),
                                                 stop=(dj == ND - 1))
                            gt = ws.tile([128, 512], BF16, tag="gt", bufs=2,
                                         name="gt")
                            nc.scalar.activation(gt[:], pg[:], AF.Silu)
                            ut = ws.tile([128, 512], BF16, tag="ut", bufs=2,
                                         name="ut")
                            nc.vector.tensor_copy(ut[:], pu[:])
                            nc.vector.tensor_tensor(H[:, mi, hs], gt[:], ut[:],
                                                    ALU.mult)
                    # ---- halo exchange ----
                    hstage = ws.tile([128, 2 * NIH], BF16, tag="hstage",
                                     name=f"hstage{half}")
                    nc.vector.tensor_copy(hstage[:, 0:NIH], H[:, :, 2])
                    nc.vector.tensor_copy(hstage[:, NIH:2 * NIH], H[:, :, 2 + T - 1])
                    nc.gpsimd.dma_start(halo_in[half][:], hstage[:])
                    nc.gpsimd.collective_compute(
                        "AllGather", ALU.bypass, replica_groups=GRP_PAIR,
                        ins=[halo_in[half].opt()], outs=[halo_out[half].opt()])
                    hg0 = ws.tile([128, 2 * NIH], BF16, tag="hg0", name="hg0")
                    nc.gpsimd.dma_start(hg0[:], halo_out[half][0])
                    hg1 = ws.tile([128, 2 * NIH], BF16, tag="hg1", name="hg1")
                    nc.gpsimd.dma_start(hg1[:], halo_out[half][1])
                    halL = ws.tile([128, NIH], BF16, tag="halL", name="halL")
                    nc.vector.tensor_scalar(halL[:], hg0[:, NIH:2 * NIH],
                                            lrsel_sb[:, 0:1], None, ALU.mult)
                    nc.vector.scalar_tensor_tensor(halL[:], hg1[:, NIH:2 * NIH],
                                                   lrsel_sb[:, 1:2], halL[:],
                                                   ALU.mult, ALU.add)
                    halR = ws.tile([128, NIH], BF16, tag="halR", name="halR")
                    nc.vector.tensor_scalar(halR[:], hg0[:, 0:NIH],
                                            lrsel_sb[:, 2:3], None, ALU.mult)
                    nc.vector.scalar_tensor_tensor(halR[:], hg1[:, 0:NIH],
                                                   lrsel_sb[:, 3:4], halR[:],
                                                   ALU.mult, ALU.add)
                    nc.vector.tensor_copy(H[:, :, 1], halL[:])
                    nc.vector.tensor_copy(H[:, :, 2 + T], halR[:])
                    # ---- depthwise conv along tokens ----
                    for mi in range(NIH):
                        w0 = dwk_sb[:, ibase + mi, 0:1]
                        wm = dwk_sb[:, ibase + mi, 1:2]
                        w2_ = dwk_sb[:, ibase + mi, 2:3]
                        tcv = ws.tile([128, T], BF16, tag="tcv", bufs=2, name="tcv")
                        nc.vector.tensor_scalar(tcv[:], H[:, mi, 1:1 + T], w0, None,
                                                ALU.mult)
                        nc.vector.scalar_tensor_tensor(tcv[:], H[:, mi, 2:2 + T], wm,
                                                       tcv[:], ALU.mult, ALU.add)
                        nc.vector.scalar_tensor_tensor(tcv[:], H[:, mi, 3:3 + T],
                                                       w2_, tcv[:], ALU.mult,
                                                       ALU.add)
                        nc.vector.tensor_copy(H[:, mi, 2:2 + T], tcv[:])
                    # ---- W_down GEMM ----
                    for dmq in range(4):
                        ns = slice(dmq * 256, (dmq + 1) * 256)
                        wd = ws.tile([128, NIH, 256], BF16, tag="wd", bufs=2,
                                     name="wd")
                        nc.gpsimd.dma_start(
                            wd[:],
                            wdownT[half * 2048:(half + 1) * 2048, ns]
                            .rearrange("(mi p) n -> p mi n", p=128))
                        for mt in range(NT):
                            ms = slice(2 + mt * 128, 2 + (mt + 1) * 128)
                            po = ps7.tile([128, 256], F32, tag="wdp", bufs=4,
                                          name="po")
                            for mi in range(NIH):
                                nc.tensor.matmul(po[:], H[:, mi, ms], wd[:, mi, :],
                                                 start=(mi == 0),
                                                 stop=(mi == NIH - 1))
                            if half == 0:
                                nc.scalar.activation(acc[:, mt, ns], po[:], AF.Copy)
                            else:
                                accf = ws.tile([128, 256], F32, tag="accf", bufs=2,
                                               name="accf")
                                nc.scalar.activation(accf[:], acc[:, mt, ns],
                                                     AF.Copy)
                                s1 = ws.tile([128, 256], F32, tag="s1", bufs=2,
                                             name="s1")
                                nc.vector.tensor_tensor(s1[:], po[:], accf[:],
                                                        ALU.add)
                                qiv = ws.tile([128, 256], BF16, tag="qiv", bufs=2,
                                              name="qiv")
                                nc.sync.dma_start(
                                    qiv[:], qint_sp[mt * 128:(mt + 1) * 128, ns])
                                qif = ws.tile([128, 256], F32, tag="qif", bufs=2,
                                              name="qif")
                                nc.scalar.activation(qif[:], qiv[:], AF.Copy)
                                ot = ws.tile([128, 256], F32, tag="ot", bufs=2,
                                             name="ot")
                                nc.vector.tensor_tensor(ot[:], s1[:], qif[:],
                                                        ALU.add)
                                nc.sync.dma_start(
                                    out_ext[mt * 128:(mt + 1) * 128, ns], ot[:])

    nc.compile()
    return nc


def _build_cached(dt_safe, lam_safe):
    key = (round(float(dt_safe), 8), round(float(lam_safe), 8))
    if key not in _cache:
        _cache[key] = build(float(dt_safe), float(lam_safe))
    return _cache[key]


def kernel(**inputs):
    _install_ntff_shim()
    Q_in = np.ascontiguousarray(inputs["Q_in"], dtype=np.float32)
    X = np.ascontiguousarray(inputs["X"], dtype=np.float32)
    cos = np.ascontiguousarray(inputs["cos"], dtype=np.float32)
    sin = np.ascontiguousarray(inputs["sin"], dtype=np.float32)
    hyper_q_w = np.asarray(inputs["hyper_q_w"], dtype=np.float32)
    hyper_k_w = np.asarray(inputs["hyper_k_w"], dtype=np.float32)
    B_Q = np.asarray(inputs["B_Q"], dtype=np.float32)
    B_K = np.asarray(inputs["B_K"], dtype=np.float32)
    W_up = np.asarray(inputs["W_up"], dtype=np.float32)
    dw = np.asarray(inputs["dw_conv_w"], dtype=np.float32)
    W_down = np.asarray(inputs["W_down"], dtype=np.float32)
    m_proj_w = np.asarray(inputs["m_proj_w"], dtype=np.float32)
    m_proj_b = np.asarray(inputs["m_proj_b"], dtype=np.float32)
    n1g = np.asarray(inputs["norm1_g"], dtype=np.float32)
    n1b = np.asarray(inputs["norm1_b"], dtype=np.float32)
    n2g = np.asarray(inputs["norm2_g"], dtype=np.float32)
    n2b = np.asarray(inputs["norm2_b"], dtype=np.float32)
    dt = float(np.asarray(inputs["dt"]))
    lam = float(np.asarray(inputs["lam"]))

    dt_safe = float(np.log1p(np.exp(dt)))
    lam_safe = float(np.log1p(np.exp(lam)))

    nc = _build_cached(dt_safe, lam_safe)

    hyperT = np.stack([np.ascontiguousarray(hyper_q_w.T),
                       np.ascontiguousarray(hyper_k_w.T)])  # [2, DM, DM*DS]
    wupT = np.ascontiguousarray(W_up.T)
    wdownT = np.ascontiguousarray(W_down.T)
    mprojT = np.ascontiguousarray(m_proj_w.T)
    nrm_bc = np.stack([
        np.broadcast_to(n1g, (128, DM)),
        np.broadcast_to(n1b, (128, DM)),
        np.broadcast_to(n2g, (128, DM)),
        np.broadcast_to(n2b, (128, DM)),
        np.broadcast_to(dt_safe * m_proj_b, (128, DM)),
    ]).astype(np.float32)
    bias_q = np.stack([B_Q, -B_Q], axis=1).astype(np.float32)
    bias_k_bc = np.ascontiguousarray(np.broadcast_to(B_K, (128, DS)),
                                     dtype=np.float32)
    dwk = np.ascontiguousarray(
        dw[:, 0, :].reshape(2 * NIH, 128, 3).transpose(1, 0, 2),
        dtype=np.float32)

    in_maps = []
    for c in range(NCORES):
        b, h = c // 2, c % 2
        tok = slice(h * T, (h + 1) * T)
        rsl = slice(c * RS, (c + 1) * RS)
        bsel = np.zeros((128, B), np.float32)
        bsel[:, b] = 1.0
        lrsel = np.zeros((128, 4), np.float32)
        if h == 0:
            lrsel[:, 3] = 1.0  # right halo comes from pair-shard 1
        else:
            lrsel[:, 0] = 1.0  # left halo comes from pair-shard 0
        in_maps.append({
            "q_in": np.ascontiguousarray(Q_in[b, tok]),
            "x_in": np.ascontiguousarray(X[b, tok]),
            "cosn": np.ascontiguousarray(cos[tok]),
            "sinn": np.ascontiguousarray(sin[tok]),
            "hqk": np.ascontiguousarray(hyperT[:, :, rsl]),
            "wupT": wupT,
            "wdownT": wdownT,
            "mprojT": mprojT,
            "nrm_bc": nrm_bc,
            "bias_q": bias_q,
            "bias_k_bc": bias_k_bc,
            "dwk": dwk,
            "bsel": bsel,
            "lrsel": lrsel,
        })

    trace = bool(os.environ.get("BASS_KERNEL_TRACE"))
    res = run_bass_kernel_spmd(nc, in_maps, core_ids=list(range(NCORES)),
                               trace=trace,
                               tmpdir=os.environ.get("BASS_KERNEL_TMPDIR"))
    kernel._last = res

    out = np.empty((B, N, DM), np.float32)
    for c in range(NCORES):
        b, h = c // 2, c % 2
        out[b, h * T:(h + 1) * T] = res.results[c]["out"]
    return out


# revision 9
# speedup vs baseline: 1.2635x; 1.2635x over previous
"""Trainium2 Bass kernel for nn_AMK_Block (sparse_attention), 8 NeuronCores.

Sharding: core c => (batch b = c//2, seq half h = c%2), T=2048 tokens/core.
Collectives: q_pool AllReduce (8 cores), Omega AllGather (8, hyper GEMM
row-sharded), C/phi_k_sum pair AllReduce, conv-halo pair AllGather.
Heavy GEMMs in bf16 (fp32 accumulate); norms/elementwise mostly fp32.
"""
import os
import sys
import types
import numpy as np

import concourse.bass as bass
import concourse.mybir as mybir
import concourse.tile as tile
from concourse import bacc
from concourse.bass_utils import run_bass_kernel_spmd
from concourse.masks import make_identity

F32 = mybir.dt.float32
BF16 = mybir.dt.bfloat16
AF = mybir.ActivationFunctionType
ALU = mybir.AluOpType
AX = mybir.AxisListType

NCORES = 8
B, N, DM, DS = 4, 4096, 1024, 64
INNER = 4 * DM
T = N // 2               # tokens per core
NT = T // 128            # 16 tok tiles
ND = DM // 128           # 8 d tiles
NIH = INNER // 128 // 2  # 16 i-tiles per inner half
RS = DM * DS // NCORES   # 8192 hyper rows per core
TPAD = T + 4             # H free dim with halo pad (cols 1..2050 used)

_cache = {}


def _install_ntff_shim():
    if "antenv.axon_hooks" in sys.modules:
        return
    try:
        from trn_agent_boot.trn_boot import _ntff_profile_via_ctypes
        hook = _ntff_profile_via_ctypes("/opt/axon/libaxon_pjrt.so")
    except Exception:
        hook = None
    m = types.ModuleType("antenv.axon_hooks")
    m.get_axon_ntff_profile_hook = lambda: hook
    m.set_axon_ntff_profile_hook = lambda h: None
    sys.modules["antenv.axon_hooks"] = m


def build(dt_safe: float, lam_safe: float):
    nc = bacc.Bacc(None, target_bir_lowering=False, debug=False)

    q_in = nc.dram_tensor("q_in", [T, DM], F32, kind="ExternalInput")
    x_in = nc.dram_tensor("x_in", [T, DM], F32, kind="ExternalInput")
    cosn = nc.dram_tensor("cosn", [T, DM], F32, kind="ExternalInput")
    sinn = nc.dram_tensor("sinn", [T, DM], F32, kind="ExternalInput")
    hqk = nc.dram_tensor("hqk", [2, DM, RS], F32, kind="ExternalInput")
    wupT = nc.dram_tensor("wupT", [DM, 2 * INNER], F32, kind="ExternalInput")
    wdownT = nc.dram_tensor("wdownT", [INNER, DM], F32, kind="ExternalInput")
    mprojT = nc.dram_tensor("mprojT", [DM, DM], F32, kind="ExternalInput")
    nrm_bc = nc.dram_tensor("nrm_bc", [5, 128, DM], F32, kind="ExternalInput")
    bias_q = nc.dram_tensor("bias_q", [DS, 2], F32, kind="ExternalInput")
    bias_k_bc = nc.dram_tensor("bias_k_bc", [128, DS], F32, kind="ExternalInput")
    dwk = nc.dram_tensor("dwk", [128, 2 * NIH, 3], F32, kind="ExternalInput")
    bsel = nc.dram_tensor("bsel", [128, B], F32, kind="ExternalInput")
    lrsel = nc.dram_tensor("lrsel", [128, 4], F32, kind="ExternalInput")
    out_ext = nc.dram_tensor("out", [T, DM], F32, kind="ExternalOutput")

    GRP_ALL = [list(range(NCORES))]
    GRP_PAIR = [[2 * i, 2 * i + 1] for i in range(B)]

    def ln_tile(pool, stats, xf, g_bc, b_bc, eps_ap):
        """LayerNorm over free dim of xf [128, DM] f32; returns f32 tile."""
        smu = stats.tile([128, 1], F32, tag="smu", bufs=3, name="smu")
        nc.vector.tensor_reduce(smu[:], xf[:], AX.X, ALU.add)
        negmu = stats.tile([128, 1], F32, tag="negmu", bufs=3, name="negmu")
        nc.vector.tensor_scalar(negmu[:], smu[:], -1.0 / DM, None, ALU.mult)
        xc = pool.tile([128, DM], F32, tag="ln_xc", bufs=2, name="ln_xc")
        nc.vector.tensor_scalar(xc[:], xf[:], negmu[:], None, ALU.add)
        sq = pool.tile([128, DM], F32, tag="ln_sq", bufs=2, name="ln_sq")
        vs = stats.tile([128, 1], F32, tag="vs", bufs=3, name="vs")
        nc.scalar.activation(sq[:], xc[:], AF.Square, accum_out=vs[:])
        std = stats.tile([128, 1], F32, tag="std", bufs=3, name="std")
        nc.scalar.activation(std[:], vs[:], AF.Sqrt, bias=eps_ap, scale=1.0 / DM)
        rs = stats.tile([128, 1], F32, tag="rs", bufs=3, name="rs")
        nc.vector.reciprocal(rs[:], std[:])
        xn = pool.tile([128, DM], F32, tag="ln_xn", bufs=2, name="ln_xn")
        nc.vector.tensor_scalar(xn[:], xc[:], rs[:], None, ALU.mult)
        nc.vector.tensor_tensor(xc[:], xn[:], g_bc[:], ALU.mult)
        lnout = pool.tile([128, DM], F32, tag="ln_out", bufs=2, name="ln_out")
        nc.vector.tensor_tensor(lnout[:], xc[:], b_bc[:], ALU.add)
        return lnout

    def pe_transpose_tile(pspool, src_b16, dstT, r, ident_b):
        """Transpose [128, DM] bf16 -> dstT[:, dj, r] for dj in 0..ND-1 via PE."""
        for g in range(2):  # 4 d-blocks per psum tile
            ptr = pspool.tile([128, 512], BF16, tag="tr", bufs=2, name="ptr")
            for k in range(4):
                dj = g * 4 + k
                nc.tensor.matmul(ptr[:, k * 128:(k + 1) * 128],
                                 src_b16[:, dj * 128:(dj + 1) * 128], ident_b[:],
                                 is_transpose=True, start=True, stop=True)
            nc.vector.tensor_copy(dstT[:, g * 4:(g + 1) * 4, r], ptr[:])

    with tile.TileContext(nc) as tc:
        with tc.tile_pool(name="dram", bufs=1, space="DRAM") as dram, \
             tc.tile_pool(name="const", bufs=1) as constp, \
             tc.tile_pool(name="keep", bufs=1) as keep, \
             tc.tile_pool(name="stats", bufs=1) as stats:

            # ---------------- DRAM scratch ----------------
            def dt_(shape, dtype, nm, shared=False):
                return dram.tile(shape, dtype, tag=nm, name=nm,
                                 addr_space="Shared" if shared else "Local")
            qp_in = dt_([B, DM], F32, "qp_in")
            qp_out = dt_([B, DM], F32, "qp_out", True)
            om_in = dt_([2, B, RS], F32, "om_in")
            om_out = dt_([NCORES, 2, B, RS], F32, "om_out", True)
            c_in = dt_([DS, DM + 1], F32, "c_in")
            c_out = dt_([DS, DM + 1], F32, "c_out")
            qn1_sp = dt_([T, DM], BF16, "qn1_sp")
            base_sp = dt_([T, DM], BF16, "base_sp")
            qint_sp = dt_([T, DM], BF16, "qint_sp")
            acc_sp = dt_([T, DM], BF16, "acc_sp")
            halo_in = [dt_([128, 2 * NIH], BF16, f"halo_in{k}") for k in range(2)]
            halo_out = [dt_([2, 128, 2 * NIH], BF16, f"halo_out{k}")
                        for k in range(2)]

            # ---------------- constants ----------------
            ones_b = constp.tile([128, 1], BF16, tag="ones_b", name="ones_b")
            nc.vector.memset(ones_b[:], 1.0)
            eps_c = constp.tile([128, 1], F32, tag="eps_c", name="eps_c")
            nc.vector.memset(eps_c[:], 1e-5)
            ident = constp.tile([128, 128], F32, tag="ident", name="ident")
            make_identity(nc, ident)
            ident_b = constp.tile([128, 128], BF16, tag="ident_b", name="ident_b")
            make_identity(nc, ident_b)
            bq = constp.tile([DS, 2], F32, tag="bq", name="bq")
            nc.sync.dma_start(bq[:], bias_q[:])
            bk_bc = constp.tile([128, DS], F32, tag="bk_bc", name="bk_bc")
            nc.sync.dma_start(bk_bc[:], bias_k_bc[:])
            dwk_sb = constp.tile([128, 2 * NIH, 3], F32, tag="dwk", name="dwk_sb")
            nc.sync.dma_start(dwk_sb[:], dwk[:])
            dwk_b = constp.tile([128, 2 * NIH, 3], BF16, tag="dwk_b", name="dwk_b")
            nc.gpsimd.dma_start(dwk_b[:], dwk[:])
            bsel_sb = constp.tile([128, B], F32, tag="bsel", name="bsel_sb")
            nc.sync.dma_start(bsel_sb[:], bsel[:])
            lrsel_sb = constp.tile([128, 4], F32, tag="lrsel", name="lrsel_sb")
            nc.sync.dma_start(lrsel_sb[:], lrsel[:])
            n2g_bc = constp.tile([128, DM], F32, tag="n2g", name="n2g_bc")
            nc.sync.dma_start(n2g_bc[:], nrm_bc[2])
            n2b_bc = constp.tile([128, DM], F32, tag="n2b", name="n2b_bc")
            nc.sync.dma_start(n2b_bc[:], nrm_bc[3])

            # long-lived across phases
            qn2T = keep.tile([128, ND, T], BF16, tag="qn2T", name="qn2T")
            rN = keep.tile([128, NT], F32, tag="rN", name="rN")

            with tc.tile_pool(name="transp", bufs=1) as transp:
                qrotT = transp.tile([128, ND, T], BF16, tag="tbig", name="qrotT")

                # ============ phase 1 ============
                with tc.tile_pool(name="w1", bufs=1) as w1, \
                     tc.tile_pool(name="ps1", bufs=1, space="PSUM") as ps1:
                    n1g_bc = w1.tile([128, DM], F32, tag="n1g", name="n1g_bc")
                    nc.sync.dma_start(n1g_bc[:], nrm_bc[0])
                    n1b_bc = w1.tile([128, DM], F32, tag="n1b", name="n1b_bc")
                    nc.sync.dma_start(n1b_bc[:], nrm_bc[1])
                    bb_bc = w1.tile([128, DM], F32, tag="bb", name="bb_bc")
                    nc.sync.dma_start(bb_bc[:], nrm_bc[4])
                    psqp = [ps1.tile([1, 512], F32, tag="qp", bufs=2, name=f"psqp{k}")
                            for k in range(2)]

                    for i in range(NT):
                        r = slice(i * 128, (i + 1) * 128)
                        qt = w1.tile([128, DM], F32, tag="qt", bufs=2, name="qt")
                        nc.sync.dma_start(qt[:], q_in[r, :])
                        xt = w1.tile([128, DM], F32, tag="xt", bufs=2, name="xt")
                        nc.sync.dma_start(xt[:], x_in[r, :])
                        ct = w1.tile([128, DM], F32, tag="ct", bufs=2, name="ct")
                        nc.sync.dma_start(ct[:], cosn[r, :])
                        st = w1.tile([128, DM], F32, tag="st", bufs=2, name="st")
                        nc.sync.dma_start(st[:], sinn[r, :])

                        qn1f = ln_tile(w1, stats, qt, n1g_bc, n1b_bc, eps_c[:])
                        qn1b = w1.tile([128, DM], BF16, tag="qn1b", bufs=2,
                                       name="qn1b")
                        nc.scalar.activation(qn1b[:], qn1f[:], AF.Copy)
                        nc.sync.dma_start(qn1_sp[r, :], qn1b[:])

                        for hf in range(2):
                            cs = slice(hf * 512, (hf + 1) * 512)
                            nc.tensor.matmul(psqp[hf][:], ones_b[:], qn1b[:, cs],
                                             start=(i == 0), stop=(i == NT - 1))

                        t1 = w1.tile([128, DM], F32, tag="t1", bufs=2, name="t1")
                        nc.vector.tensor_tensor(t1[:], qn1f[:], ct[:], ALU.mult)
                        u1 = w1.tile([128, 512], F32, tag="u1", bufs=2, name="u1")
                        nc.vector.tensor_tensor(u1[:], qn1f[:, 512:], st[:, :512],
                                                ALU.mult)
                        qr = w1.tile([128, DM], BF16, tag="qr", bufs=2, name="qr")
                        nc.vector.tensor_tensor(qr[:, :512], t1[:, :512], u1[:],
                                                ALU.subtract)
                        u2 = w1.tile([128, 512], F32, tag="u2", bufs=2, name="u2")
                        nc.vector.tensor_tensor(u2[:], qn1f[:, :512], st[:, 512:],
                                                ALU.mult)
                        nc.vector.tensor_tensor(qr[:, 512:], t1[:, 512:], u2[:],
                                                ALU.add)
                        pe_transpose_tile(ps1, qr, qrotT, r, ident_b)

                        bt = w1.tile([128, DM], F32, tag="bt", bufs=2, name="bt")
                        nc.scalar.activation(bt[:], qt[:], AF.Copy,
                                             scale=1.0 - lam_safe)
                        btx = w1.tile([128, DM], F32, tag="btx", bufs=2, name="btx")
                        nc.scalar.activation(btx[:], xt[:], AF.Copy, scale=lam_safe)
                        nc.vector.tensor_tensor(bt[:], bt[:], btx[:], ALU.add)
                        baseb = w1.tile([128, DM], BF16, tag="baseb", bufs=2,
                                        name="baseb")
                        nc.vector.tensor_tensor(baseb[:], bt[:], bb_bc[:], ALU.add)
                        nc.sync.dma_start(base_sp[r, :], baseb[:])

                    # ---- q_pool allreduce ----
                    qp_stage = w1.tile([1, DM], F32, tag="qp_stage", name="qp_stage")
                    z4 = w1.tile([B, DM], F32, tag="z4", name="z4")
                    for hf in range(2):
                        cs = slice(hf * 512, (hf + 1) * 512)
                        nc.scalar.activation(qp_stage[:, cs], psqp[hf][:], AF.Copy,
                                             scale=1.0 / N)
                        ps4 = ps1.tile([B, 512], F32, tag="b4", bufs=2, name="ps4")
                        nc.tensor.matmul(ps4[:], bsel_sb[0:1, :], qp_stage[:, cs],
                                         start=True, stop=True)
                        nc.scalar.activation(z4[:, cs], ps4[:], AF.Copy)
                    nc.gpsimd.dma_start(qp_in[:], z4[:])
                    nc.gpsimd.collective_compute(
                        "AllReduce", ALU.add, replica_groups=GRP_ALL,
                        ins=[qp_in.opt()], outs=[qp_out.opt()])

                # ============ phase 2: hyper GEMM + Omega allgather ============
                with tc.tile_pool(name="w2", bufs=1) as w2, \
                     tc.tile_pool(name="ps2", bufs=1, space="PSUM") as ps2:
                    qp4 = w2.tile([B, DM], F32, tag="qp4", name="qp4")
                    nc.gpsimd.dma_start(qp4[:], qp_out[:])
                    qpT = w2.tile([128, ND, B], BF16, tag="qpT", name="qpT")
                    for dj in range(ND):
                        pst = ps2.tile([128, B], F32, tag="tp", bufs=2, name="pst")
                        nc.tensor.transpose(pst[:], qp4[:, dj * 128:(dj + 1) * 128],
                                            ident[0:B, 0:B])
                        nc.scalar.activation(qpT[:, dj, :], pst[:], AF.Copy)

                    for mat in range(2):
                        for rc in range(RS // 512):
                            rcs = slice(rc * 512, (rc + 1) * 512)
                            hk = w2.tile([128, ND, 512], BF16, tag="hk", bufs=3,
                                         name="hk")
                            nc.gpsimd.dma_start(
                                hk[:],
                                hqk[mat, :, rcs].rearrange("(dj p) r -> p dj r",
                                                           p=128))
                            pso = ps2.tile([B, 512], F32, tag="b4", bufs=2,
                                           name="pso")
                            for dj in range(ND):
                                nc.tensor.matmul(pso[:], qpT[:, dj, :], hk[:, dj, :],
                                                 start=(dj == 0),
                                                 stop=(dj == ND - 1))
                            st512 = w2.tile([B, 512], F32, tag="st512", bufs=3,
                                            name="st512")
                            nc.scalar.activation(st512[:], pso[:], AF.Copy)
                            nc.sync.dma_start(om_in[mat, :, rcs], st512[:])
                    nc.gpsimd.collective_compute(
                        "AllGather", ALU.bypass, replica_groups=GRP_ALL,
                        ins=[om_in.opt()], outs=[om_out.opt()])

                # ============ phases 3-5 ============
                with tc.tile_pool(name="w3", bufs=1) as w3:
                    om_sb = w3.tile([128, 2, ND, DS], BF16, tag="om_sb",
                                    name="om_sb")
                    for mat in range(2):
                        for dj in range(ND):
                            for b in range(B):
                                obt = w3.tile([128, DS], BF16, tag="obt", bufs=4,
                                              name="obt")
                                nc.gpsimd.dma_start(
                                    obt[:],
                                    om_out[dj, mat, b].rearrange("(p e) -> p e",
                                                                 p=128))
                                if b == 0:
                                    nc.vector.tensor_scalar(
                                        om_sb[:, mat, dj, :], obt[:],
                                        bsel_sb[:, 0:1], None, ALU.mult)
                                else:
                                    nc.vector.scalar_tensor_tensor(
                                        om_sb[:, mat, dj, :], obt[:],
                                        bsel_sb[:, b:b + 1], om_sb[:, mat, dj, :],
                                        ALU.mult, ALU.add)

                    phiK = w3.tile([128, NT, DS], BF16, tag="phiK", name="phiK")
                    phiQT = w3.tile([DS, NT, 128], BF16, tag="phiQT", name="phiQT")
                    with tc.tile_pool(name="ps3", bufs=1, space="PSUM") as ps3:
                        for i in range(NT):
                            r = slice(i * 128, (i + 1) * 128)
                            pk = ps3.tile([128, DS], F32, tag="phi", bufs=2,
                                          name="pk")
                            for dj in range(ND):
                                nc.tensor.matmul(pk[:], qrotT[:, dj, r],
                                                 om_sb[:, 1, dj, :],
                                                 start=(dj == 0),
                                                 stop=(dj == ND - 1))
                            zf = w3.tile([128, DS], F32, tag="zf", bufs=2, name="zf")
                            nc.vector.tensor_tensor(zf[:], pk[:], bk_bc[:], ALU.add)
                            rf = w3.tile([128, DS], F32, tag="rf", bufs=2, name="rf")
                            nc.scalar.activation(rf[:], zf[:], AF.Relu)
                            ef = w3.tile([128, DS], F32, tag="ef", bufs=2, name="ef")
                            nc.vector.tensor_tensor(ef[:], zf[:], rf[:],
                                                    ALU.subtract)
                            nc.scalar.activation(ef[:], ef[:], AF.Exp)
                            nc.vector.tensor_tensor(phiK[:, i, :], rf[:], ef[:],
                                                    ALU.add)

                            pq = ps3.tile([DS, 128], F32, tag="phiq", bufs=2,
                                          name="pq")
                            for dj in range(ND):
                                nc.tensor.matmul(pq[:], om_sb[:, 0, dj, :],
                                                 qrotT[:, dj, r],
                                                 start=(dj == 0),
                                                 stop=(dj == ND - 1))
                            t1q = w3.tile([DS, 128], F32, tag="t1q", bufs=2,
                                          name="t1q")
                            nc.scalar.activation(t1q[:], pq[:], AF.Relu,
                                                 bias=bq[:, 0:1])
                            t2q = w3.tile([DS, 128], F32, tag="t2q", bufs=2,
                                          name="t2q")
                            nc.scalar.activation(t2q[:], pq[:], AF.Relu,
                                                 bias=bq[:, 1:2], scale=-1.0)
                            nc.scalar.activation(t2q[:], t2q[:], AF.Exp, scale=-1.0)
                            nc.vector.tensor_tensor(phiQT[:, i, :], t1q[:], t2q[:],
                                                    ALU.add)

                        # ---- C GEMM + phi_k_sum + pair allreduce ----
                        psc = [ps3.tile([DS, 512], F32, tag="c", bufs=2,
                                        name=f"psc{k}") for k in range(2)]
                        psk = ps3.tile([DS, 1], F32, tag="pks", bufs=1, name="psk")
                        for i in range(NT):
                            qn1t = w3.tile([128, DM], BF16, tag="qn1t", bufs=3,
                                           name="qn1t")
                            nc.sync.dma_start(qn1t[:],
                                              qn1_sp[i * 128:(i + 1) * 128, :])
                            for nb in range(2):
                                nc.tensor.matmul(psc[nb][:], phiK[:, i, :],
                                                 qn1t[:, nb * 512:(nb + 1) * 512],
                                                 start=(i == 0), stop=(i == NT - 1))
                            nc.tensor.matmul(psk[:], phiK[:, i, :], ones_b[:],
                                             start=(i == 0), stop=(i == NT - 1))
                        c_stage = w3.tile([DS, DM + 1], F32, tag="c_stage",
                                          name="c_stage")
                        for nb in range(2):
                            nc.scalar.activation(
                                c_stage[:, nb * 512:(nb + 1) * 512], psc[nb][:],
                                AF.Copy)
                        nc.scalar.activation(c_stage[:, DM:DM + 1], psk[:], AF.Copy)
                        nc.gpsimd.dma_start(c_in[:], c_stage[:])
                        nc.gpsimd.collective_compute(
                            "AllReduce", ALU.add, replica_groups=GRP_PAIR,
                            ins=[c_in.opt()], outs=[c_out.opt()])

                    c_b = w3.tile([DS, DM], BF16, tag="c_b", name="c_b")
                    nc.gpsimd.dma_start(c_b[:], c_out[:, 0:DM])
                    pks_b = w3.tile([DS, 1], BF16, tag="pks_b", name="pks_b")
                    nc.gpsimd.dma_start(pks_b[:], c_out[:, DM:DM + 1])

                    mprojb = w3.tile([128, ND, DM], BF16, tag="mprojb",
                                     name="mprojb")
                    nc.gpsimd.dma_start(
                        mprojb[:],
                        mprojT[:, :].rearrange("(dj p) n -> p dj n", p=128))

                    # ---- phase 5 ----
                    mT = transp.tile([128, ND, T], BF16, tag="tbig", name="mT")
                    with tc.tile_pool(name="ps5", bufs=1, space="PSUM") as ps5:
                        for i in range(NT):
                            r = slice(i * 128, (i + 1) * 128)
                            pd = ps5.tile([128, 1], F32, tag="d1", bufs=2, name="pd")
                            nc.tensor.matmul(pd[:], phiQT[:, i, :], pks_b[:],
                                             start=True, stop=True)
                            ab = stats.tile([128, 1], F32, tag="ab", bufs=2,
                                            name="ab")
                            nc.scalar.activation(ab[:], pd[:], AF.Abs)
                            nc.vector.tensor_scalar(ab[:], ab[:], 1.0, None, ALU.add)
                            nc.vector.reciprocal(rN[:, i:i + 1], ab[:])

                            qn1t = w3.tile([128, DM], BF16, tag="qn1t", bufs=3,
                                           name="qn1t")
                            nc.sync.dma_start(qn1t[:], qn1_sp[r, :])
                            mb = w3.tile([128, DM], BF16, tag="mb", bufs=2,
                                         name="mb")
                            for nb in range(2):
                                cs = slice(nb * 512, (nb + 1) * 512)
                                pa = ps5.tile([128, 512], F32, tag="mm512", bufs=4,
                                              name="pa")
                                nc.tensor.matmul(pa[:], phiQT[:, i, :], c_b[:, cs],
                                                 start=True, stop=True)
                                tb = w3.tile([128, 512], BF16, tag="tb", bufs=2,
                                             name="tb")
                                nc.vector.tensor_scalar(tb[:], pa[:], rN[:, i:i + 1],
                                                        None, ALU.mult)
                                nc.vector.tensor_tensor(mb[:, cs], tb[:],
                                                        qn1t[:, cs], ALU.subtract)
                            pe_transpose_tile(ps5, mb, mT, r, ident_b)

                            baset = w3.tile([128, DM], BF16, tag="baset", bufs=2,
                                            name="baset")
                            nc.sync.dma_start(baset[:], base_sp[r, :])
                            qint = w3.tile([128, DM], F32, tag="qint", bufs=2,
                                           name="qint")
                            for nb in range(2):
                                cs = slice(nb * 512, (nb + 1) * 512)
                                pm = ps5.tile([128, 512], F32, tag="mm512", bufs=4,
                                              name="pm")
                                for dj in range(ND):
                                    nc.tensor.matmul(pm[:], mT[:, dj, r],
                                                     mprojb[:, dj, cs],
                                                     start=(dj == 0),
                                                     stop=(dj == ND - 1))
                                tb2 = w3.tile([128, 512], BF16, tag="tb2", bufs=2,
                                              name="tb2")
                                nc.vector.tensor_scalar(tb2[:], pm[:], dt_safe,
                                                        None, ALU.mult)
                                nc.vector.tensor_tensor(qint[:, cs], tb2[:],
                                                        baset[:, cs], ALU.add)
                            qint_b = w3.tile([128, DM], BF16, tag="qint_b", bufs=2,
                                             name="qint_b")
                            nc.gpsimd.tensor_copy(qint_b[:], qint[:])
                            nc.sync.dma_start(qint_sp[r, :], qint_b[:])
                            qn2f = ln_tile(w3, stats, qint, n2g_bc, n2b_bc, eps_c[:])
                            qn2b = w3.tile([128, DM], BF16, tag="qn2b", bufs=2,
                                           name="qn2b")
                            nc.scalar.activation(qn2b[:], qn2f[:], AF.Copy)
                            pe_transpose_tile(ps5, qn2b, qn2T, r, ident_b)

            # ============ phases 6-7: FFN ============
            with tc.tile_pool(name="ffn", bufs=1) as ffn, \
                 tc.tile_pool(name="ws", bufs=1) as ws:
                H = ffn.tile([128, NIH, TPAD], BF16, tag="H", name="H")
                for half in range(2):
                    ibase = half * NIH
                    sv = ws.tile([128, NIH, 4], BF16, tag="sv", name="sv")
                    # zero halo columns (cols 0..1 and 2050..2051)
                    nc.gpsimd.memset(H[:, :, 0:2], 0.0)
                    nc.gpsimd.memset(H[:, :, 2 + T:4 + T], 0.0)
                    with tc.tile_pool(name="psgu", bufs=1, space="PSUM") as psgu:
                        for mi in range(NIH):
                            g0 = half * 2048 + mi * 128
                            u0 = INNER + g0
                            wu_g = ws.tile([128, ND, 128], BF16, tag="wu_g", bufs=3,
                                           name="wu_g")
                            nc.gpsimd.dma_start(
                                wu_g[:],
                                wupT[:, g0:g0 + 128].rearrange(
                                    "(dj p) f -> p dj f", p=128))
                            wu_u = ws.tile([128, ND, 128], BF16, tag="wu_u", bufs=3,
                                           name="wu_u")
                            nc.gpsimd.dma_start(
                                wu_u[:],
                                wupT[:, u0:u0 + 128].rearrange(
                                    "(dj p) f -> p dj f", p=128))
                            for nb in range(2):
                                cs0 = nb * 1024
                                hs = slice(2 + cs0, 2 + cs0 + 1024)
                                pg = psgu.tile([128, 1024], F32, tag="pg", bufs=2,
                                               name="pg")
                                pu = psgu.tile([128, 1024], F32, tag="pu", bufs=2,
                                               name="pu")
                                for hb in range(2):
                                    cs = slice(cs0 + hb * 512, cs0 + (hb + 1) * 512)
                                    ps_s = slice(hb * 512, (hb + 1) * 512)
                                    for dj in range(ND):
                                        nc.tensor.matmul(pg[:, ps_s], wu_g[:, dj, :],
                                                         qn2T[:, dj, cs],
                                                         start=(dj == 0),
                                                         stop=(dj == ND - 1))
                                    for dj in range(ND):
                                        nc.tensor.matmul(pu[:, ps_s], wu_u[:, dj, :],
                                                         qn2T[:, dj, cs],
                                                         start=(dj == 0),
                                                         stop=(dj == ND - 1))
                                gt = ws.tile([128, 1024], BF16, tag="gt", bufs=2,
                                             name="gt")
                                nc.scalar.activation(gt[:], pg[:], AF.Silu)
                                ut = ws.tile([128, 1024], BF16, tag="ut", bufs=2,
                                             name="ut")
                                nc.vector.tensor_copy(ut[:], pu[:])
                                nc.vector.tensor_tensor(H[:, mi, hs], gt[:], ut[:],
                                                        ALU.mult)
                            # save pre-conv boundary cols, then conv (interior ok,
                            # boundary out-cols 2 / 2049 patched post-halo)
                            nc.gpsimd.tensor_copy(sv[:, mi, 0:2], H[:, mi, 2:4])
                            nc.gpsimd.tensor_copy(sv[:, mi, 2:4],
                                                  H[:, mi, T:T + 2])
                            w0 = dwk_sb[:, ibase + mi, 0:1]
                            wm = dwk_sb[:, ibase + mi, 1:2]
                            w2_ = dwk_sb[:, ibase + mi, 2:3]
                            tcv = ws.tile([128, T], BF16, tag="tcv", bufs=2,
                                          name="tcv")
                            nc.vector.tensor_scalar(tcv[:], H[:, mi, 1:1 + T], w0,
                                                    None, ALU.mult)
                            nc.vector.scalar_tensor_tensor(tcv[:], H[:, mi, 2:2 + T],
                                                           wm, tcv[:], ALU.mult,
                                                           ALU.add)
                            nc.vector.scalar_tensor_tensor(tcv[:], H[:, mi, 3:3 + T],
                                                           w2_, tcv[:], ALU.mult,
                                                           ALU.add)
                            nc.vector.tensor_copy(H[:, mi, 2:2 + T], tcv[:])
                    # ---- halo exchange + boundary patch ----
                    hstage = ws.tile([128, 2 * NIH], BF16, tag="hstage",
                                     name="hstage")
                    nc.vector.tensor_copy(hstage[:, 0:NIH], sv[:, :, 0])
                    nc.vector.tensor_copy(hstage[:, NIH:2 * NIH], sv[:, :, 3])
                    nc.gpsimd.dma_start(halo_in[half][:], hstage[:])
                    nc.gpsimd.collective_compute(
                        "AllGather", ALU.bypass, replica_groups=GRP_PAIR,
                        ins=[halo_in[half].opt()], outs=[halo_out[half].opt()])
                    hg0 = ws.tile([128, 2 * NIH], BF16, tag="hg0", name="hg0")
                    nc.gpsimd.dma_start(hg0[:], halo_out[half][0])
                    hg1 = ws.tile([128, 2 * NIH], BF16, tag="hg1", name="hg1")
                    nc.gpsimd.dma_start(hg1[:], halo_out[half][1])
                    halL = ws.tile([128, NIH], BF16, tag="halL", name="halL")
                    nc.vector.tensor_scalar(halL[:], hg0[:, NIH:2 * NIH],
                                            lrsel_sb[:, 0:1], None, ALU.mult)
                    nc.vector.scalar_tensor_tensor(halL[:], hg1[:, NIH:2 * NIH],
                                                   lrsel_sb[:, 1:2], halL[:],
                                                   ALU.mult, ALU.add)
                    halR = ws.tile([128, NIH], BF16, tag="halR", name="halR")
                    nc.vector.tensor_scalar(halR[:], hg0[:, 0:NIH],
                                            lrsel_sb[:, 2:3], None, ALU.mult)
                    nc.vector.scalar_tensor_tensor(halR[:], hg1[:, 0:NIH],
                                                   lrsel_sb[:, 3:4], halR[:],
                                                   ALU.mult, ALU.add)
                    # patch out-col 2:  w0*halL + w1*sv0 + w2*sv1
                    w0v = dwk_b[:, ibase:ibase + NIH, 0]
                    w1v = dwk_b[:, ibase:ibase + NIH, 1]
                    w2v = dwk_b[:, ibase:ibase + NIH, 2]
                    pt1 = ws.tile([128, NIH], BF16, tag="pt1", name="pt1")
                    pt2 = ws.tile([128, NIH], BF16, tag="pt2", name="pt2")
                    nc.vector.tensor_tensor(pt1[:], halL[:], w0v, ALU.mult)
                    nc.vector.tensor_tensor(pt2[:], sv[:, :, 0], w1v, ALU.mult)
                    nc.vector.tensor_tensor(pt1[:], pt1[:], pt2[:], ALU.add)
                    nc.vector.tensor_tensor(pt2[:], sv[:, :, 1], w2v, ALU.mult)
                    nc.vector.tensor_tensor(H[:, :, 2], pt1[:], pt2[:], ALU.add)
                    # patch out-col 2049: w0*sv2 + w1*sv3 + w2*halR
                    nc.vector.tensor_tensor(pt1[:], sv[:, :, 2], w0v, ALU.mult)
                    nc.vector.tensor_tensor(pt2[:], sv[:, :, 3], w1v, ALU.mult)
                    nc.vector.tensor_tensor(pt1[:], pt1[:], pt2[:], ALU.add)
                    nc.vector.tensor_tensor(pt2[:], halR[:], w2v, ALU.mult)
                    nc.vector.tensor_tensor(H[:, :, 2 + T - 1], pt1[:], pt2[:],
                                            ALU.add)
                    # ---- W_down GEMM (dm in 2 halves of 512) ----
                    with tc.tile_pool(name="pswd", bufs=1, space="PSUM") as pswd:
                        for dmq in range(2):
                            ns = slice(dmq * 512, (dmq + 1) * 512)
                            wd = ws.tile([128, NIH, 512], BF16, tag="wd", bufs=2,
                                         name="wd")
                            nc.gpsimd.dma_start(
                                wd[:],
                                wdownT[half * 2048:(half + 1) * 2048, ns]
                                .rearrange("(mi p) n -> p mi n", p=128))
                            for mt in range(NT):
                                ms = slice(2 + mt * 128, 2 + (mt + 1) * 128)
                                rr = slice(mt * 128, (mt + 1) * 128)
                                po = pswd.tile([128, 512], F32, tag="wdp", bufs=4,
                                               name="po")
                                for mi in range(NIH):
                                    nc.tensor.matmul(po[:], H[:, mi, ms],
                                                     wd[:, mi, :],
                                                     start=(mi == 0),
                                                     stop=(mi == NIH - 1))
                                if half == 0:
                                    qiv = ws.tile([128, 512], BF16, tag="qiv",
                                                  bufs=2, name="qiv")
                                    nc.sync.dma_start(qiv[:], qint_sp[rr, ns])
                                    qif = ws.tile([128, 512], F32, tag="qif",
                                                  bufs=2, name="qif")
                                    nc.gpsimd.tensor_copy(qif[:], qiv[:])
                                    a0 = ws.tile([128, 512], BF16, tag="a0",
                                                 bufs=2, name="a0")
                                    nc.vector.tensor_tensor(a0[:], po[:], qif[:],
                                                            ALU.add)
                                    nc.sync.dma_start(acc_sp[rr, ns], a0[:])
                                else:
                                    av = ws.tile([128, 512], BF16, tag="av",
                                                 bufs=2, name="av")
                                    nc.sync.dma_start(av[:], acc_sp[rr, ns])
                                    af = ws.tile([128, 512], F32, tag="af",
                                                 bufs=2, name="af")
                                    nc.gpsimd.tensor_copy(af[:], av[:])
                                    ot = ws.tile([128, 512], F32, tag="ot",
                                                 bufs=2, name="ot")
                                    nc.vector.tensor_tensor(ot[:], po[:], af[:],
                                                            ALU.add)
                                    nc.sync.dma_start(out_ext[rr, ns], ot[:])

    nc.compile()
    return nc


def _build_cached(dt_safe, lam_safe):
    key = (round(float(dt_safe), 8), round(float(lam_safe), 8))
    if key not in _cache:
        _cache[key] = build(float(dt_safe), float(lam_safe))
    return _cache[key]


def kernel(**inputs):
    _install_ntff_shim()
    Q_in = np.ascontiguousarray(inputs["Q_in"], dtype=np.float32)
    X = np.ascontiguousarray(inputs["X"], dtype=np.float32)
    cos = np.ascontiguousarray(inputs["cos"], dtype=np.float32)
    sin = np.ascontiguousarray(inputs["sin"], dtype=np.float32)
    hyper_q_w = np.asarray(inputs["hyper_q_w"], dtype=np.float32)
    hyper_k_w = np.asarray(inputs["hyper_k_w"], dtype=np.float32)
    B_Q = np.asarray(inputs["B_Q"], dtype=np.float32)
    B_K = np.asarray(inputs["B_K"], dtype=np.float32)
    W_up = np.asarray(inputs["W_up"], dtype=np.float32)
    dw = np.asarray(inputs["dw_conv_w"], dtype=np.float32)
    W_down = np.asarray(inputs["W_down"], dtype=np.float32)
    m_proj_w = np.asarray(inputs["m_proj_w"], dtype=np.float32)
    m_proj_b = np.asarray(inputs["m_proj_b"], dtype=np.float32)
    n1g = np.asarray(inputs["norm1_g"], dtype=np.float32)
    n1b = np.asarray(inputs["norm1_b"], dtype=np.float32)
    n2g = np.asarray(inputs["norm2_g"], dtype=np.float32)
    n2b = np.asarray(inputs["norm2_b"], dtype=np.float32)
    dt = float(np.asarray(inputs["dt"]))
    lam = float(np.asarray(inputs["lam"]))

    dt_safe = float(np.log1p(np.exp(dt)))
    lam_safe = float(np.log1p(np.exp(lam)))

    nc = _build_cached(dt_safe, lam_safe)

    hyperT = np.stack([np.ascontiguousarray(hyper_q_w.T),
                       np.ascontiguousarray(hyper_k_w.T)])  # [2, DM, DM*DS]
    wupT = np.ascontiguousarray(W_up.T)
    wdownT = np.ascontiguousarray(W_down.T)
    mprojT = np.ascontiguousarray(m_proj_w.T)
    nrm_bc = np.stack([
        np.broadcast_to(n1g, (128, DM)),
        np.broadcast_to(n1b, (128, DM)),
        np.broadcast_to(n2g, (128, DM)),
        np.broadcast_to(n2b, (128, DM)),
        np.broadcast_to(dt_safe * m_proj_b, (128, DM)),
    ]).astype(np.float32)
    bias_q = np.stack([B_Q, -B_Q], axis=1).astype(np.float32)
    bias_k_bc = np.ascontiguousarray(np.broadcast_to(B_K, (128, DS)),
                                     dtype=np.float32)
    dwk = np.ascontiguousarray(
        dw[:, 0, :].reshape(2 * NIH, 128, 3).transpose(1, 0, 2),
        dtype=np.float32)

    in_maps = []
    for c in range(NCORES):
        b, h = c // 2, c % 2
        tok = slice(h * T, (h + 1) * T)
        rsl = slice(c * RS, (c + 1) * RS)
        bsel = np.zeros((128, B), np.float32)
        bsel[:, b] = 1.0
        lrsel = np.zeros((128, 4), np.float32)
        if h == 0:
            lrsel[:, 3] = 1.0  # right halo comes from pair-shard 1
        else:
            lrsel[:, 0] = 1.0  # left halo comes from pair-shard 0
        in_maps.append({
            "q_in": np.ascontiguousarray(Q_in[b, tok]),
            "x_in": np.ascontiguousarray(X[b, tok]),
            "cosn": np.ascontiguousarray(cos[tok]),
            "sinn": np.ascontiguousarray(sin[tok]),
            "hqk": np.ascontiguousarray(hyperT[:, :, rsl]),
            "wupT": wupT,
            "wdownT": wdownT,
            "mprojT": mprojT,
            "nrm_bc": nrm_bc,
            "bias_q": bias_q,
            "bias_k_bc": bias_k_bc,
            "dwk": dwk,
            "bsel": bsel,
            "lrsel": lrsel,
        })

    trace = bool(os.environ.get("BASS_KERNEL_TRACE"))
    res = run_bass_kernel_spmd(nc, in_maps, core_ids=list(range(NCORES)),
                               trace=trace,
                               tmpdir=os.environ.get("BASS_KERNEL_TMPDIR"))
    kernel._last = res

    out = np.empty((B, N, DM), np.float32)
    for c in range(NCORES):
        b, h = c // 2, c % 2
        out[b, h * T:(h + 1) * T] = res.results[c]["out"]
    return out


# revision 10
# speedup vs baseline: 1.4119x; 1.1174x over previous
"""Trainium2 Bass kernel for nn_AMK_Block (sparse_attention), 8 NeuronCores.

Sharding: core c => (batch b = c//2, seq half h = c%2), T=2048 tokens/core.
Collectives: q_pool AllReduce (8 cores), Omega AllGather (8, hyper GEMM
row-sharded), C/phi_k_sum pair AllReduce, conv-halo pair AllGather.
Heavy GEMMs in bf16 (fp32 accumulate); norms/elementwise mostly fp32.
This build specializes for identity norm affine (g==1, b==0) and
m_proj_b==0; kernel() verifies and falls back to a general build.
"""
import os
import sys
import types
import numpy as np
import ml_dtypes

import concourse.bass as bass
import concourse.mybir as mybir
import concourse.tile as tile
from concourse import bacc
from concourse.bass_utils import run_bass_kernel_spmd
from concourse.masks import make_identity

F32 = mybir.dt.float32
BF16 = mybir.dt.bfloat16
AF = mybir.ActivationFunctionType
ALU = mybir.AluOpType
AX = mybir.AxisListType

NCORES = 8
B, N, DM, DS = 4, 4096, 1024, 64
INNER = 4 * DM
T = N // 2               # tokens per core
NT = T // 128            # 16 tok tiles
ND = DM // 128           # 8 d tiles
NIH = INNER // 128 // 2  # 16 i-tiles per inner half
RS = DM * DS // NCORES   # 8192 hyper rows per core
TPAD = T + 4             # H free dim with halo pad (cols 1..2050 used)

_cache = {}


def _install_ntff_shim():
    if "antenv.axon_hooks" in sys.modules:
        return
    try:
        from trn_agent_boot.trn_boot import _ntff_profile_via_ctypes
        hook = _ntff_profile_via_ctypes("/opt/axon/libaxon_pjrt.so")
    except Exception:
        hook = None
    m = types.ModuleType("antenv.axon_hooks")
    m.get_axon_ntff_profile_hook = lambda: hook
    m.set_axon_ntff_profile_hook = lambda h: None
    sys.modules["antenv.axon_hooks"] = m


def build(dt_safe: float, lam_safe: float, ident_norm: bool):
    """ident_norm=True assumes norm g==1/b==0 and m_proj_b==0 (host-checked)."""
    nc = bacc.Bacc(None, target_bir_lowering=False, debug=False)

    q_in = nc.dram_tensor("q_in", [T, DM], F32, kind="ExternalInput")
    base_in = nc.dram_tensor("base_in", [T, DM], F32, kind="ExternalInput")
    cosn = nc.dram_tensor("cosn", [T, DM], F32, kind="ExternalInput")
    sinn = nc.dram_tensor("sinn", [T, DM], F32, kind="ExternalInput")
    hqk = nc.dram_tensor("hqk", [2, DM, RS], BF16, kind="ExternalInput")
    wupT = nc.dram_tensor("wupT", [DM, 2 * INNER], F32, kind="ExternalInput")
    wdownT = nc.dram_tensor("wdownT", [INNER, DM], F32, kind="ExternalInput")
    mprojT = nc.dram_tensor("mprojT", [DM, DM], F32, kind="ExternalInput")
    nrm_bc = nc.dram_tensor("nrm_bc", [4, 128, DM], F32, kind="ExternalInput")
    bias_q = nc.dram_tensor("bias_q", [DS, 2], F32, kind="ExternalInput")
    bias_k_bc = nc.dram_tensor("bias_k_bc", [128, DS], F32, kind="ExternalInput")
    dwk = nc.dram_tensor("dwk", [128, 2 * NIH, 3], F32, kind="ExternalInput")
    bsel = nc.dram_tensor("bsel", [128, B], F32, kind="ExternalInput")
    lrsel = nc.dram_tensor("lrsel", [128, 4], F32, kind="ExternalInput")
    out_ext = nc.dram_tensor("out", [T, DM], F32, kind="ExternalOutput")

    GRP_ALL = [list(range(NCORES))]
    GRP_PAIR = [[2 * i, 2 * i + 1] for i in range(B)]

    def ln_tile(pool, stats, xf, g_bc, b_bc, eps_ap, sqb=1):
        """LayerNorm over free dim of xf [128, DM] f32; returns f32 tile."""
        smu = stats.tile([128, 1], F32, tag="smu", bufs=3, name="smu")
        nc.vector.tensor_reduce(smu[:], xf[:], AX.X, ALU.add)
        negmu = stats.tile([128, 1], F32, tag="negmu", bufs=3, name="negmu")
        nc.vector.tensor_scalar(negmu[:], smu[:], -1.0 / DM, None, ALU.mult)
        xc = pool.tile([128, DM], F32, tag="ln_xc", bufs=2, name="ln_xc")
        nc.vector.tensor_scalar(xc[:], xf[:], negmu[:], None, ALU.add)
        sq = pool.tile([128, DM], F32, tag="ln_sq", bufs=sqb, name="ln_sq")
        vs = stats.tile([128, 1], F32, tag="vs", bufs=3, name="vs")
        nc.scalar.activation(sq[:], xc[:], AF.Square, accum_out=vs[:])
        std = stats.tile([128, 1], F32, tag="std", bufs=3, name="std")
        nc.scalar.activation(std[:], vs[:], AF.Sqrt, bias=eps_ap, scale=1.0 / DM)
        rs = stats.tile([128, 1], F32, tag="rs", bufs=3, name="rs")
        nc.vector.reciprocal(rs[:], std[:])
        xn = pool.tile([128, DM], F32, tag="ln_xn", bufs=2, name="ln_xn")
        nc.vector.tensor_scalar(xn[:], xc[:], rs[:], None, ALU.mult)
        if ident_norm:
            return xn
        nc.vector.tensor_tensor(xc[:], xn[:], g_bc[:], ALU.mult)
        lnout = pool.tile([128, DM], F32, tag="ln_out", bufs=2, name="ln_out")
        nc.vector.tensor_tensor(lnout[:], xc[:], b_bc[:], ALU.add)
        return lnout

    def pe_transpose_tile(pspool, src_b16, dstT, r, ident_b):
        """Transpose [128, DM] bf16 -> dstT[:, dj, r] for dj in 0..ND-1 via PE."""
        for g in range(2):
            ptr = pspool.tile([128, 512], BF16, tag="tr", bufs=2, name="ptr")
            for k in range(4):
                dj = g * 4 + k
                nc.tensor.matmul(ptr[:, k * 128:(k + 1) * 128],
                                 src_b16[:, dj * 128:(dj + 1) * 128], ident_b[:],
                                 is_transpose=True, start=True, stop=True)
            nc.vector.tensor_copy(dstT[:, g * 4:(g + 1) * 4, r], ptr[:])

    with tile.TileContext(nc) as tc:
        with tc.tile_pool(name="dram", bufs=1, space="DRAM") as dram, \
             tc.tile_pool(name="const", bufs=1) as constp, \
             tc.tile_pool(name="keep", bufs=1) as keep, \
             tc.tile_pool(name="stats", bufs=1) as stats:

            # ---------------- DRAM scratch ----------------
            def dt_(shape, dtype, nm, shared=False):
                return dram.tile(shape, dtype, tag=nm, name=nm,
                                 addr_space="Shared" if shared else "Local")
            qp_in = dt_([B, DM], F32, "qp_in")
            qp_out = dt_([B, DM], F32, "qp_out", True)
            om_in = dt_([2, B, RS], F32, "om_in")
            om_out = dt_([NCORES, 2, B, RS], F32, "om_out", True)
            c_in = dt_([DS, DM + 1], F32, "c_in")
            c_out = dt_([DS, DM + 1], F32, "c_out")
            qn1_sp = dt_([T, DM], BF16, "qn1_sp")
            qint_sp = dt_([T, DM], BF16, "qint_sp")
            acc_sp = dt_([T, DM], BF16, "acc_sp")
            halo_in = [dt_([128, 2 * NIH], BF16, f"halo_in{k}") for k in range(2)]
            halo_out = [dt_([2, 128, 2 * NIH], BF16, f"halo_out{k}")
                        for k in range(2)]

            # ---------------- constants ----------------
            ones_b = constp.tile([128, 1], BF16, tag="ones_b", name="ones_b")
            nc.vector.memset(ones_b[:], 1.0)
            eps_c = constp.tile([128, 1], F32, tag="eps_c", name="eps_c")
            nc.vector.memset(eps_c[:], 1e-5)
            ident = constp.tile([128, 128], F32, tag="ident", name="ident")
            make_identity(nc, ident)
            ident_b = constp.tile([128, 128], BF16, tag="ident_b", name="ident_b")
            make_identity(nc, ident_b)
            bq = constp.tile([DS, 2], F32, tag="bq", name="bq")
            nc.sync.dma_start(bq[:], bias_q[:])
            bk_bc = constp.tile([128, DS], F32, tag="bk_bc", name="bk_bc")
            nc.sync.dma_start(bk_bc[:], bias_k_bc[:])
            dwk_sb = constp.tile([128, 2 * NIH, 3], F32, tag="dwk", name="dwk_sb")
            nc.sync.dma_start(dwk_sb[:], dwk[:])
            dwk_b = constp.tile([128, 2 * NIH, 3], BF16, tag="dwk_b", name="dwk_b")
            nc.gpsimd.dma_start(dwk_b[:], dwk[:])
            bsel_sb = constp.tile([128, B], F32, tag="bsel", name="bsel_sb")
            nc.sync.dma_start(bsel_sb[:], bsel[:])
            lrsel_sb = constp.tile([128, 4], F32, tag="lrsel", name="lrsel_sb")
            nc.sync.dma_start(lrsel_sb[:], lrsel[:])
            if ident_norm:
                n1g_bc = n1b_bc = n2g_bc = n2b_bc = None
            else:
                n1g_bc = keep.tile([128, DM], F32, tag="n1g", name="n1g_bc")
                nc.sync.dma_start(n1g_bc[:], nrm_bc[0])
                n1b_bc = keep.tile([128, DM], F32, tag="n1b", name="n1b_bc")
                nc.sync.dma_start(n1b_bc[:], nrm_bc[1])
                n2g_bc = keep.tile([128, DM], F32, tag="n2g", name="n2g_bc")
                nc.sync.dma_start(n2g_bc[:], nrm_bc[2])
                n2b_bc = keep.tile([128, DM], F32, tag="n2b", name="n2b_bc")
                nc.sync.dma_start(n2b_bc[:], nrm_bc[3])

            # long-lived across phases
            qn2T = keep.tile([128, ND, T], BF16, tag="qn2T", name="qn2T")
            rN = keep.tile([128, NT], F32, tag="rN", name="rN")

            with tc.tile_pool(name="transp", bufs=1) as transp:
                qrotT = transp.tile([128, ND, T], BF16, tag="tbig", name="qrotT")

                # ============ phase 1: LN1, q_pool, RoPE ============
                with tc.tile_pool(name="w1", bufs=1) as w1, \
                     tc.tile_pool(name="ps1", bufs=1, space="PSUM") as ps1:
                    psqp = [ps1.tile([1, 512], F32, tag="qp", bufs=2, name=f"psqp{k}")
                            for k in range(2)]

                    for i in range(NT):
                        r = slice(i * 128, (i + 1) * 128)
                        qt = w1.tile([128, DM], F32, tag="qt", bufs=2, name="qt")
                        nc.sync.dma_start(qt[:], q_in[r, :])
                        ct = w1.tile([128, DM], F32, tag="ct", bufs=2, name="ct")
                        nc.sync.dma_start(ct[:], cosn[r, :])
                        st = w1.tile([128, DM], F32, tag="st", bufs=2, name="st")
                        nc.sync.dma_start(st[:], sinn[r, :])

                        qn1f = ln_tile(w1, stats, qt, n1g_bc, n1b_bc, eps_c[:])
                        qn1b = w1.tile([128, DM], BF16, tag="qn1b", bufs=2,
                                       name="qn1b")
                        nc.scalar.activation(qn1b[:], qn1f[:], AF.Copy)
                        nc.sync.dma_start(qn1_sp[r, :], qn1b[:])

                        for hf in range(2):
                            cs = slice(hf * 512, (hf + 1) * 512)
                            nc.tensor.matmul(psqp[hf][:], ones_b[:], qn1b[:, cs],
                                             start=(i == 0), stop=(i == NT - 1))

                        t1 = w1.tile([128, DM], F32, tag="t1", bufs=2, name="t1")
                        nc.vector.tensor_tensor(t1[:], qn1f[:], ct[:], ALU.mult)
                        u1 = w1.tile([128, 512], F32, tag="u1", bufs=2, name="u1")
                        nc.vector.tensor_tensor(u1[:], qn1f[:, 512:], st[:, :512],
                                                ALU.mult)
                        qr = w1.tile([128, DM], BF16, tag="qr", bufs=2, name="qr")
                        nc.vector.tensor_tensor(qr[:, :512], t1[:, :512], u1[:],
                                                ALU.subtract)
                        u2 = w1.tile([128, 512], F32, tag="u2", bufs=2, name="u2")
                        nc.vector.tensor_tensor(u2[:], qn1f[:, :512], st[:, 512:],
                                                ALU.mult)
                        nc.vector.tensor_tensor(qr[:, 512:], t1[:, 512:], u2[:],
                                                ALU.add)
                        pe_transpose_tile(ps1, qr, qrotT, r, ident_b)

                    # ---- q_pool allreduce ----
                    qp_stage = w1.tile([1, DM], F32, tag="qp_stage", name="qp_stage")
                    z4 = w1.tile([B, DM], F32, tag="z4", name="z4")
                    for hf in range(2):
                        cs = slice(hf * 512, (hf + 1) * 512)
                        nc.scalar.activation(qp_stage[:, cs], psqp[hf][:], AF.Copy,
                                             scale=1.0 / N)
                        ps4 = ps1.tile([B, 512], F32, tag="b4", bufs=2, name="ps4")
                        nc.tensor.matmul(ps4[:], bsel_sb[0:1, :], qp_stage[:, cs],
                                         start=True, stop=True)
                        nc.scalar.activation(z4[:, cs], ps4[:], AF.Copy)
                    nc.gpsimd.dma_start(qp_in[:], z4[:])
                    nc.gpsimd.collective_compute(
                        "AllReduce", ALU.add, replica_groups=GRP_ALL,
                        ins=[qp_in.opt()], outs=[qp_out.opt()])

                # ============ phase 2: hyper GEMM + Omega allgather ============
                with tc.tile_pool(name="w2", bufs=1) as w2, \
                     tc.tile_pool(name="ps2", bufs=1, space="PSUM") as ps2:
                    qp4 = w2.tile([B, DM], F32, tag="qp4", name="qp4")
                    nc.gpsimd.dma_start(qp4[:], qp_out[:])
                    qpT = w2.tile([128, ND, B], BF16, tag="qpT", name="qpT")
                    for dj in range(ND):
                        pst = ps2.tile([128, B], F32, tag="tp", bufs=2, name="pst")
                        nc.tensor.transpose(pst[:], qp4[:, dj * 128:(dj + 1) * 128],
                                            ident[0:B, 0:B])
                        nc.scalar.activation(qpT[:, dj, :], pst[:], AF.Copy)

                    for mat in range(2):
                        for rc in range(RS // 512):
                            rcs = slice(rc * 512, (rc + 1) * 512)
                            hk = w2.tile([128, ND, 512], BF16, tag="hk", bufs=4,
                                         name="hk")
                            nc.sync.dma_start(
                                hk[:],
                                hqk[mat, :, rcs].rearrange("(dj p) r -> p dj r",
                                                           p=128))
                            pso = ps2.tile([B, 512], F32, tag="b4", bufs=4,
                                           name="pso")
                            for dj in range(ND):
                                nc.tensor.matmul(pso[:], qpT[:, dj, :], hk[:, dj, :],
                                                 start=(dj == 0),
                                                 stop=(dj == ND - 1))
                            st512 = w2.tile([B, 512], F32, tag="st512", bufs=4,
                                            name="st512")
                            nc.vector.tensor_copy(st512[:], pso[:])
                            nc.sync.dma_start(om_in[mat, :, rcs], st512[:])
                    nc.gpsimd.collective_compute(
                        "AllGather", ALU.bypass, replica_groups=GRP_ALL,
                        ins=[om_in.opt()], outs=[om_out.opt()])

                # ============ phases 3-4 ============
                with tc.tile_pool(name="k35", bufs=1) as k35:
                    om_sb = k35.tile([128, 2, ND, DS], BF16, tag="om_sb",
                                     name="om_sb")
                    phiK = k35.tile([128, NT, DS], BF16, tag="phiK", name="phiK")
                    phiQT = k35.tile([DS, NT, 128], BF16, tag="phiQT", name="phiQT")
                    c_b = k35.tile([DS, DM], BF16, tag="c_b", name="c_b")
                    pks_b = k35.tile([DS, 1], BF16, tag="pks_b", name="pks_b")
                    mprojb = k35.tile([128, ND, DM], BF16, tag="mprojb",
                                      name="mprojb")
                    nc.gpsimd.dma_start(
                        mprojb[:],
                        mprojT[:, :].rearrange("(dj p) n -> p dj n", p=128))

                    with tc.tile_pool(name="w34", bufs=1) as w34, \
                         tc.tile_pool(name="ps3", bufs=1, space="PSUM") as ps3:
                        for mat in range(2):
                            for dj in range(ND):
                                for b in range(B):
                                    obt = w34.tile([128, DS], BF16, tag="obt",
                                                   bufs=4, name="obt")
                                    nc.gpsimd.dma_start(
                                        obt[:],
                                        om_out[dj, mat, b].rearrange(
                                            "(p e) -> p e", p=128))
                                    if b == 0:
                                        nc.vector.tensor_scalar(
                                            om_sb[:, mat, dj, :], obt[:],
                                            bsel_sb[:, 0:1], None, ALU.mult)
                                    else:
                                        nc.vector.scalar_tensor_tensor(
                                            om_sb[:, mat, dj, :], obt[:],
                                            bsel_sb[:, b:b + 1],
                                            om_sb[:, mat, dj, :],
                                            ALU.mult, ALU.add)

                        for i in range(NT):
                            r = slice(i * 128, (i + 1) * 128)
                            pk = ps3.tile([128, DS], F32, tag="phi", bufs=2,
                                          name="pk")
                            for dj in range(ND):
                                nc.tensor.matmul(pk[:], qrotT[:, dj, r],
                                                 om_sb[:, 1, dj, :],
                                                 start=(dj == 0),
                                                 stop=(dj == ND - 1))
                            zf = w34.tile([128, DS], F32, tag="zf", bufs=2,
                                          name="zf")
                            nc.vector.tensor_tensor(zf[:], pk[:], bk_bc[:], ALU.add)
                            rf = w34.tile([128, DS], F32, tag="rf", bufs=2,
                                          name="rf")
                            nc.vector.tensor_scalar(rf[:], zf[:], 0.0, None,
                                                    ALU.max)
                            ef = w34.tile([128, DS], F32, tag="ef", bufs=2,
                                          name="ef")
                            nc.vector.tensor_tensor(ef[:], zf[:], rf[:],
                                                    ALU.subtract)
                            nc.scalar.activation(ef[:], ef[:], AF.Exp)
                            nc.vector.tensor_tensor(phiK[:, i, :], rf[:], ef[:],
                                                    ALU.add)

                            pq = ps3.tile([DS, 128], F32, tag="phiq", bufs=2,
                                          name="pq")
                            for dj in range(ND):
                                nc.tensor.matmul(pq[:], om_sb[:, 0, dj, :],
                                                 qrotT[:, dj, r],
                                                 start=(dj == 0),
                                                 stop=(dj == ND - 1))
                            zq = w34.tile([DS, 128], F32, tag="zq", bufs=2,
                                          name="zq")
                            nc.vector.tensor_scalar(zq[:], pq[:], bq[:, 0:1], None,
                                                    ALU.add)
                            rq = w34.tile([DS, 128], F32, tag="rq", bufs=2,
                                          name="rq")
                            nc.vector.tensor_scalar(rq[:], zq[:], 0.0, None,
                                                    ALU.max)
                            eq = w34.tile([DS, 128], F32, tag="eq", bufs=2,
                                          name="eq")
                            nc.vector.tensor_tensor(eq[:], zq[:], rq[:],
                                                    ALU.subtract)
                            nc.scalar.activation(eq[:], eq[:], AF.Exp)
                            nc.vector.tensor_tensor(phiQT[:, i, :], rq[:], eq[:],
                                                    ALU.add)

                        # ---- C GEMM + phi_k_sum + pair allreduce ----
                        psc = [ps3.tile([DS, 512], F32, tag="c", bufs=2,
                                        name=f"psc{k}") for k in range(2)]
                        psk = ps3.tile([DS, 1], F32, tag="pks", bufs=1, name="psk")
                        for i in range(NT):
                            qn1t = w34.tile([128, DM], BF16, tag="qn1t", bufs=3,
                                            name="qn1t")
                            nc.sync.dma_start(qn1t[:],
                                              qn1_sp[i * 128:(i + 1) * 128, :])
                            for nb in range(2):
                                nc.tensor.matmul(psc[nb][:], phiK[:, i, :],
                                                 qn1t[:, nb * 512:(nb + 1) * 512],
                                                 start=(i == 0), stop=(i == NT - 1))
                            nc.tensor.matmul(psk[:], phiK[:, i, :], ones_b[:],
                                             start=(i == 0), stop=(i == NT - 1))
                        c_stage = w34.tile([DS, DM + 1], F32, tag="c_stage",
                                           name="c_stage")
                        for nb in range(2):
                            nc.vector.tensor_copy(
                                c_stage[:, nb * 512:(nb + 1) * 512], psc[nb][:])
                        nc.vector.tensor_copy(c_stage[:, DM:DM + 1], psk[:])
                        nc.gpsimd.dma_start(c_in[:], c_stage[:])
                        nc.gpsimd.collective_compute(
                            "AllReduce", ALU.add, replica_groups=GRP_PAIR,
                            ins=[c_in.opt()], outs=[c_out.opt()])

                        nc.gpsimd.dma_start(c_b[:], c_out[:, 0:DM])
                        nc.gpsimd.dma_start(pks_b[:], c_out[:, DM:DM + 1])

                    # ============ phase 5 ============
                    mT = transp.tile([128, ND, T], BF16, tag="tbig", name="mT")
                    with tc.tile_pool(name="w5", bufs=1) as w5, \
                         tc.tile_pool(name="ps5", bufs=1, space="PSUM") as ps5:
                        for i in range(NT):
                            r = slice(i * 128, (i + 1) * 128)
                            pd = ps5.tile([128, 1], F32, tag="d1", bufs=2, name="pd")
                            nc.tensor.matmul(pd[:], phiQT[:, i, :], pks_b[:],
                                             start=True, stop=True)
                            ab = stats.tile([128, 1], F32, tag="ab", bufs=2,
                                            name="ab")
                            nc.scalar.activation(ab[:], pd[:], AF.Abs)
                            nc.vector.tensor_scalar(ab[:], ab[:], 1.0, None,
                                                    ALU.add)
                            nc.vector.reciprocal(rN[:, i:i + 1], ab[:])

                            qn1t = w5.tile([128, DM], F32, tag="qn1tf", bufs=2,
                                           name="qn1tf")
                            nc.gpsimd.dma_start(qn1t[:], qn1_sp[r, :])
                            mb = w5.tile([128, DM], BF16, tag="mb", bufs=2,
                                         name="mb")
                            for nb in range(2):
                                cs = slice(nb * 512, (nb + 1) * 512)
                                pa = ps5.tile([128, 512], F32, tag="mm512", bufs=4,
                                              name="pa")
                                nc.tensor.matmul(pa[:], phiQT[:, i, :], c_b[:, cs],
                                                 start=True, stop=True)
                                nc.vector.scalar_tensor_tensor(
                                    mb[:, cs], pa[:], rN[:, i:i + 1], qn1t[:, cs],
                                    ALU.mult, ALU.subtract)
                            pe_transpose_tile(ps5, mb, mT, r, ident_b)

                            baset = w5.tile([128, DM], F32, tag="baset", bufs=2,
                                            name="baset")
                            nc.sync.dma_start(baset[:], base_in[r, :])
                            qint = w5.tile([128, DM], F32, tag="qint", bufs=1,
                                           name="qint")
                            for nb in range(2):
                                cs = slice(nb * 512, (nb + 1) * 512)
                                pm = ps5.tile([128, 512], F32, tag="mm512", bufs=4,
                                              name="pm")
                                for dj in range(ND):
                                    nc.tensor.matmul(pm[:], mT[:, dj, r],
                                                     mprojb[:, dj, cs],
                                                     start=(dj == 0),
                                                     stop=(dj == ND - 1))
                                nc.vector.scalar_tensor_tensor(
                                    qint[:, cs], pm[:], dt_safe, baset[:, cs],
                                    ALU.mult, ALU.add)
                            qint_b = w5.tile([128, DM], BF16, tag="qint_b", bufs=2,
                                             name="qint_b")
                            nc.scalar.activation(qint_b[:], qint[:], AF.Copy)
                            nc.sync.dma_start(qint_sp[r, :], qint_b[:])
                            qn2f = ln_tile(w5, stats, qint, n2g_bc, n2b_bc,
                                           eps_c[:])
                            qn2b = w5.tile([128, DM], BF16, tag="qn2b", bufs=2,
                                           name="qn2b")
                            nc.scalar.activation(qn2b[:], qn2f[:], AF.Copy)
                            pe_transpose_tile(ps5, qn2b, qn2T, r, ident_b)

            # ============ phases 6-7: FFN ============
            with tc.tile_pool(name="ffn", bufs=1) as ffn, \
                 tc.tile_pool(name="ws", bufs=1) as ws:
                H = ffn.tile([128, NIH, TPAD], BF16, tag="H", name="H")
                for half in range(2):
                    ibase = half * NIH
                    sv = ws.tile([128, NIH, 4], BF16, tag="sv", name="sv")
                    nc.gpsimd.memset(H[:, :, 0:2], 0.0)
                    nc.gpsimd.memset(H[:, :, 2 + T:4 + T], 0.0)
                    with tc.tile_pool(name="psgu", bufs=1, space="PSUM") as psgu:
                        for mi in range(NIH):
                            g0 = half * 2048 + mi * 128
                            u0 = INNER + g0
                            wu_g = ws.tile([128, ND, 128], BF16, tag="wu_g", bufs=3,
                                           name="wu_g")
                            nc.gpsimd.dma_start(
                                wu_g[:],
                                wupT[:, g0:g0 + 128].rearrange(
                                    "(dj p) f -> p dj f", p=128))
                            wu_u = ws.tile([128, ND, 128], BF16, tag="wu_u", bufs=3,
                                           name="wu_u")
                            nc.gpsimd.dma_start(
                                wu_u[:],
                                wupT[:, u0:u0 + 128].rearrange(
                                    "(dj p) f -> p dj f", p=128))
                            for nb in range(2):
                                cs0 = nb * 1024
                                hs = slice(2 + cs0, 2 + cs0 + 1024)
                                pg = psgu.tile([128, 1024], F32, tag="pg", bufs=2,
                                               name="pg")
                                pu = psgu.tile([128, 1024], F32, tag="pu", bufs=2,
                                               name="pu")
                                for hb in range(2):
                                    cs = slice(cs0 + hb * 512, cs0 + (hb + 1) * 512)
                                    ps_s = slice(hb * 512, (hb + 1) * 512)
                                    for dj in range(ND):
                                        nc.tensor.matmul(pg[:, ps_s], wu_g[:, dj, :],
                                                         qn2T[:, dj, cs],
                                                         start=(dj == 0),
                                                         stop=(dj == ND - 1))
                                    for dj in range(ND):
                                        nc.tensor.matmul(pu[:, ps_s], wu_u[:, dj, :],
                                                         qn2T[:, dj, cs],
                                                         start=(dj == 0),
                                                         stop=(dj == ND - 1))
                                gt = ws.tile([128, 1024], F32, tag="gt", bufs=2,
                                             name="gt")
                                nc.scalar.activation(gt[:], pg[:], AF.Silu)
                                nc.vector.tensor_tensor(H[:, mi, hs], gt[:], pu[:],
                                                        ALU.mult)
                            nc.gpsimd.tensor_copy(sv[:, mi, 0:2], H[:, mi, 2:4])
                            nc.gpsimd.tensor_copy(sv[:, mi, 2:4],
                                                  H[:, mi, T:T + 2])
                            w0 = dwk_sb[:, ibase + mi, 0:1]
                            wm = dwk_sb[:, ibase + mi, 1:2]
                            w2_ = dwk_sb[:, ibase + mi, 2:3]
                            tcv = ws.tile([128, T], BF16, tag="tcv", bufs=2,
                                          name="tcv")
                            nc.vector.tensor_scalar(tcv[:], H[:, mi, 1:1 + T], w0,
                                                    None, ALU.mult)
                            nc.vector.scalar_tensor_tensor(tcv[:], H[:, mi, 2:2 + T],
                                                           wm, tcv[:], ALU.mult,
                                                           ALU.add)
                            nc.vector.scalar_tensor_tensor(tcv[:], H[:, mi, 3:3 + T],
                                                           w2_, tcv[:], ALU.mult,
                                                           ALU.add)
                            nc.vector.tensor_copy(H[:, mi, 2:2 + T], tcv[:])
                    # ---- halo exchange + boundary patch ----
                    hstage = ws.tile([128, 2 * NIH], BF16, tag="hstage",
                                     name="hstage")
                    nc.vector.tensor_copy(hstage[:, 0:NIH], sv[:, :, 0])
                    nc.vector.tensor_copy(hstage[:, NIH:2 * NIH], sv[:, :, 3])
                    nc.gpsimd.dma_start(halo_in[half][:], hstage[:])
                    nc.gpsimd.collective_compute(
                        "AllGather", ALU.bypass, replica_groups=GRP_PAIR,
                        ins=[halo_in[half].opt()], outs=[halo_out[half].opt()])
                    hg0 = ws.tile([128, 2 * NIH], BF16, tag="hg0", name="hg0")
                    nc.gpsimd.dma_start(hg0[:], halo_out[half][0])
                    hg1 = ws.tile([128, 2 * NIH], BF16, tag="hg1", name="hg1")
                    nc.gpsimd.dma_start(hg1[:], halo_out[half][1])
                    halL = ws.tile([128, NIH], BF16, tag="halL", name="halL")
                    nc.vector.tensor_scalar(halL[:], hg0[:, NIH:2 * NIH],
                                            lrsel_sb[:, 0:1], None, ALU.mult)
                    nc.vector.scalar_tensor_tensor(halL[:], hg1[:, NIH:2 * NIH],
                                                   lrsel_sb[:, 1:2], halL[:],
                                                   ALU.mult, ALU.add)
                    halR = ws.tile([128, NIH], BF16, tag="halR", name="halR")
                    nc.vector.tensor_scalar(halR[:], hg0[:, 0:NIH],
                                            lrsel_sb[:, 2:3], None, ALU.mult)
                    nc.vector.scalar_tensor_tensor(halR[:], hg1[:, 0:NIH],
                                                   lrsel_sb[:, 3:4], halR[:],
                                                   ALU.mult, ALU.add)
                    w0v = dwk_b[:, ibase:ibase + NIH, 0]
                    w1v = dwk_b[:, ibase:ibase + NIH, 1]
                    w2v = dwk_b[:, ibase:ibase + NIH, 2]
                    pt1 = ws.tile([128, NIH], BF16, tag="pt1", name="pt1")
                    pt2 = ws.tile([128, NIH], BF16, tag="pt2", name="pt2")
                    nc.vector.tensor_tensor(pt1[:], halL[:], w0v, ALU.mult)
                    nc.vector.tensor_tensor(pt2[:], sv[:, :, 0], w1v, ALU.mult)
                    nc.vector.tensor_tensor(pt1[:], pt1[:], pt2[:], ALU.add)
                    nc.vector.tensor_tensor(pt2[:], sv[:, :, 1], w2v, ALU.mult)
                    nc.vector.tensor_tensor(H[:, :, 2], pt1[:], pt2[:], ALU.add)
                    nc.vector.tensor_tensor(pt1[:], sv[:, :, 2], w0v, ALU.mult)
                    nc.vector.tensor_tensor(pt2[:], sv[:, :, 3], w1v, ALU.mult)
                    nc.vector.tensor_tensor(pt1[:], pt1[:], pt2[:], ALU.add)
                    nc.vector.tensor_tensor(pt2[:], halR[:], w2v, ALU.mult)
                    nc.vector.tensor_tensor(H[:, :, 2 + T - 1], pt1[:], pt2[:],
                                            ALU.add)
                    # ---- W_down GEMM (dm in 2 halves of 512) ----
                    with tc.tile_pool(name="pswd", bufs=1, space="PSUM") as pswd:
                        for dmq in range(2):
                            ns = slice(dmq * 512, (dmq + 1) * 512)
                            wd = ws.tile([128, NIH, 512], BF16, tag="wd", bufs=2,
                                         name="wd")
                            nc.gpsimd.dma_start(
                                wd[:],
                                wdownT[half * 2048:(half + 1) * 2048, ns]
                                .rearrange("(mi p) n -> p mi n", p=128))
                            for mt in range(NT):
                                ms = slice(2 + mt * 128, 2 + (mt + 1) * 128)
                                rr = slice(mt * 128, (mt + 1) * 128)
                                po = pswd.tile([128, 512], F32, tag="wdp", bufs=4,
                                               name="po")
                                for mi in range(NIH):
                                    nc.tensor.matmul(po[:], H[:, mi, ms],
                                                     wd[:, mi, :],
                                                     start=(mi == 0),
                                                     stop=(mi == NIH - 1))
                                if half == 0:
                                    qiv = ws.tile([128, 512], BF16, tag="qiv",
                                                  bufs=2, name="qiv")
                                    nc.sync.dma_start(qiv[:], qint_sp[rr, ns])
                                    qif = ws.tile([128, 512], F32, tag="qif",
                                                  bufs=2, name="qif")
                                    nc.scalar.activation(qif[:], qiv[:], AF.Copy)
                                    a0 = ws.tile([128, 512], BF16, tag="a0",
                                                 bufs=2, name="a0")
                                    nc.vector.tensor_tensor(a0[:], po[:], qif[:],
                                                            ALU.add)
                                    nc.sync.dma_start(acc_sp[rr, ns], a0[:])
                                else:
                                    av = ws.tile([128, 512], BF16, tag="av",
                                                 bufs=2, name="av")
                                    nc.sync.dma_start(av[:], acc_sp[rr, ns])
                                    af = ws.tile([128, 512], F32, tag="af",
                                                 bufs=2, name="af")
                                    nc.scalar.activation(af[:], av[:], AF.Copy)
                                    ot = ws.tile([128, 512], F32, tag="ot",
                                                 bufs=2, name="ot")
                                    nc.vector.tensor_tensor(ot[:], po[:], af[:],
                                                            ALU.add)
                                    nc.sync.dma_start(out_ext[rr, ns], ot[:])

    nc.compile()
    return nc


def _build_cached(dt_safe, lam_safe, ident_norm):
    key = (round(float(dt_safe), 8), round(float(lam_safe), 8), bool(ident_norm))
    if key not in _cache:
        _cache[key] = build(float(dt_safe), float(lam_safe), bool(ident_norm))
    return _cache[key]


def kernel(**inputs):
    _install_ntff_shim()
    Q_in = np.ascontiguousarray(inputs["Q_in"], dtype=np.float32)
    X = np.ascontiguousarray(inputs["X"], dtype=np.float32)
    cos = np.ascontiguousarray(inputs["cos"], dtype=np.float32)
    sin = np.ascontiguousarray(inputs["sin"], dtype=np.float32)
    hyper_q_w = np.asarray(inputs["hyper_q_w"], dtype=np.float32)
    hyper_k_w = np.asarray(inputs["hyper_k_w"], dtype=np.float32)
    B_Q = np.asarray(inputs["B_Q"], dtype=np.float32)
    B_K = np.asarray(inputs["B_K"], dtype=np.float32)
    W_up = np.asarray(inputs["W_up"], dtype=np.float32)
    dw = np.asarray(inputs["dw_conv_w"], dtype=np.float32)
    W_down = np.asarray(inputs["W_down"], dtype=np.float32)
    m_proj_w = np.asarray(inputs["m_proj_w"], dtype=np.float32)
    m_proj_b = np.asarray(inputs["m_proj_b"], dtype=np.float32)
    n1g = np.asarray(inputs["norm1_g"], dtype=np.float32)
    n1b = np.asarray(inputs["norm1_b"], dtype=np.float32)
    n2g = np.asarray(inputs["norm2_g"], dtype=np.float32)
    n2b = np.asarray(inputs["norm2_b"], dtype=np.float32)
    dt = float(np.asarray(inputs["dt"]))
    lam = float(np.asarray(inputs["lam"]))

    dt_safe = float(np.log1p(np.exp(dt)))
    lam_safe = float(np.log1p(np.exp(lam)))
    ident_norm = bool(np.all(n1g == 1) and np.all(n1b == 0)
                      and np.all(n2g == 1) and np.all(n2b == 0)
                      and np.all(m_proj_b == 0))

    nc = _build_cached(dt_safe, lam_safe, ident_norm)

    # host-side sharding / layout prep
    hyperT = np.stack([hyper_q_w.T, hyper_k_w.T]).astype(ml_dtypes.bfloat16)
    wupT = np.ascontiguousarray(W_up.T)
    wdownT = np.ascontiguousarray(W_down.T)
    mprojT = np.ascontiguousarray(m_proj_w.T)
    nrm_bc = np.stack([
        np.broadcast_to(n1g, (128, DM)),
        np.broadcast_to(n1b, (128, DM)),
        np.broadcast_to(n2g, (128, DM)),
        np.broadcast_to(n2b, (128, DM)),
    ]).astype(np.float32)
    bias_q = np.stack([B_Q, -B_Q], axis=1).astype(np.float32)
    bias_k_bc = np.ascontiguousarray(np.broadcast_to(B_K, (128, DS)),
                                     dtype=np.float32)
    dwk = np.ascontiguousarray(
        dw[:, 0, :].reshape(2 * NIH, 128, 3).transpose(1, 0, 2),
        dtype=np.float32)
    base_full = ((1.0 - lam_safe) * Q_in + lam_safe * X
                 + (dt_safe * m_proj_b)[None, None, :]).astype(np.float32)

    in_maps = []
    for c in range(NCORES):
        b, h = c // 2, c % 2
        tok = slice(h * T, (h + 1) * T)
        rsl = slice(c * RS, (c + 1) * RS)
        bsel = np.zeros((128, B), np.float32)
        bsel[:, b] = 1.0
        lrsel = np.zeros((128, 4), np.float32)
        if h == 0:
            lrsel[:, 3] = 1.0
        else:
            lrsel[:, 0] = 1.0
        in_maps.append({
            "q_in": np.ascontiguousarray(Q_in[b, tok]),
            "base_in": np.ascontiguousarray(base_full[b, tok]),
            "cosn": np.ascontiguousarray(cos[tok]),
            "sinn": np.ascontiguousarray(sin[tok]),
            "hqk": np.ascontiguousarray(hyperT[:, :, rsl]),
            "wupT": wupT,
            "wdownT": wdownT,
            "mprojT": mprojT,
            "nrm_bc": nrm_bc,
            "bias_q": bias_q,
            "bias_k_bc": bias_k_bc,
            "dwk": dwk,
            "bsel": bsel,
            "lrsel": lrsel,
        })

    trace = bool(os.environ.get("BASS_KERNEL_TRACE"))
    res = run_bass_kernel_spmd(nc, in_maps, core_ids=list(range(NCORES)),
                               trace=trace,
                               tmpdir=os.environ.get("BASS_KERNEL_TMPDIR"))
    kernel._last = res

    out = np.empty((B, N, DM), np.float32)
    for c in range(NCORES):
        b, h = c // 2, c % 2
        out[b, h * T:(h + 1) * T] = res.results[c]["out"]
    return out


# revision 13
# speedup vs baseline: 1.5120x; 1.0709x over previous
"""Trainium2 Bass kernel for nn_AMK_Block (sparse_attention), 8 NeuronCores.

Sharding: core c => (batch b = c//2, seq half h = c%2), T=2048 tokens/core.
Collectives: q_pool AllReduce (8 cores), Omega AllGather (8, hyper GEMM
row-sharded), C/phi_k_sum pair AllReduce, conv-halo pair AllGather.
Heavy GEMMs in bf16 (fp32 accumulate); norms/elementwise mostly fp32.
This build specializes for identity norm affine (g==1, b==0) and
m_proj_b==0; kernel() verifies and falls back to a general build.
"""
import os
import sys
import types
import numpy as np
import ml_dtypes

import concourse.bass as bass
import concourse.mybir as mybir
import concourse.tile as tile
from concourse import bacc
from concourse.bass_utils import run_bass_kernel_spmd
from concourse.masks import make_identity

F32 = mybir.dt.float32
BF16 = mybir.dt.bfloat16
AF = mybir.ActivationFunctionType
ALU = mybir.AluOpType
AX = mybir.AxisListType

NCORES = 8
B, N, DM, DS = 4, 4096, 1024, 64
INNER = 4 * DM
T = N // 2               # tokens per core
NT = T // 128            # 16 tok tiles
ND = DM // 128           # 8 d tiles
NIH = INNER // 128 // 2  # 16 i-tiles per inner half
RS = DM * DS // NCORES   # 8192 hyper rows per core
TPAD = T                 # H holds conv output, no halo pad needed

_cache = {}


def _install_ntff_shim():
    if "antenv.axon_hooks" in sys.modules:
        return
    try:
        from trn_agent_boot.trn_boot import _ntff_profile_via_ctypes
        hook = _ntff_profile_via_ctypes("/opt/axon/libaxon_pjrt.so")
    except Exception:
        hook = None
    m = types.ModuleType("antenv.axon_hooks")
    m.get_axon_ntff_profile_hook = lambda: hook
    m.set_axon_ntff_profile_hook = lambda h: None
    sys.modules["antenv.axon_hooks"] = m


def build(dt_safe: float, lam_safe: float, ident_norm: bool):
    """ident_norm=True assumes norm g==1/b==0 and m_proj_b==0 (host-checked)."""
    nc = bacc.Bacc(None, target_bir_lowering=False, debug=False)

    q_in = nc.dram_tensor("q_in", [T, DM], F32, kind="ExternalInput")
    base_in = nc.dram_tensor("base_in", [T, DM], F32, kind="ExternalInput")
    cosn = nc.dram_tensor("cosn", [T, DM], F32, kind="ExternalInput")
    sinn = nc.dram_tensor("sinn", [T, DM], F32, kind="ExternalInput")
    hqk = nc.dram_tensor("hqk", [2, DM, RS], BF16, kind="ExternalInput")
    wupT = nc.dram_tensor("wupT", [DM, 2 * INNER], F32, kind="ExternalInput")
    wdownT = nc.dram_tensor("wdownT", [INNER, DM], F32, kind="ExternalInput")
    mprojT = nc.dram_tensor("mprojT", [DM, DM], F32, kind="ExternalInput")
    nrm_bc = nc.dram_tensor("nrm_bc", [4, 128, DM], F32, kind="ExternalInput")
    bias_q = nc.dram_tensor("bias_q", [DS, 2], F32, kind="ExternalInput")
    bias_k_bc = nc.dram_tensor("bias_k_bc", [128, DS], F32, kind="ExternalInput")
    dwk = nc.dram_tensor("dwk", [128, 2 * NIH, 3], F32, kind="ExternalInput")
    bsel = nc.dram_tensor("bsel", [128, B], F32, kind="ExternalInput")
    lrsel = nc.dram_tensor("lrsel", [128, 4], F32, kind="ExternalInput")
    out_ext = nc.dram_tensor("out", [T, DM], F32, kind="ExternalOutput")

    GRP_ALL = [list(range(NCORES))]
    GRP_PAIR = [[2 * i, 2 * i + 1] for i in range(B)]

    def ln_tile(pool, stats, xf, g_bc, b_bc, eps_ap, sqb=1):
        """LayerNorm over free dim of xf [128, DM] f32; returns f32 tile."""
        smu = stats.tile([128, 1], F32, tag="smu", bufs=3, name="smu")
        nc.vector.tensor_reduce(smu[:], xf[:], AX.X, ALU.add)
        negmu = stats.tile([128, 1], F32, tag="negmu", bufs=3, name="negmu")
        nc.vector.tensor_scalar(negmu[:], smu[:], -1.0 / DM, None, ALU.mult)
        xc = pool.tile([128, DM], F32, tag="ln_xc", bufs=2, name="ln_xc")
        nc.vector.tensor_scalar(xc[:], xf[:], negmu[:], None, ALU.add)
        sq = pool.tile([128, DM], F32, tag="ln_sq", bufs=sqb, name="ln_sq")
        vs = stats.tile([128, 1], F32, tag="vs", bufs=3, name="vs")
        nc.scalar.activation(sq[:], xc[:], AF.Square, accum_out=vs[:])
        std = stats.tile([128, 1], F32, tag="std", bufs=3, name="std")
        nc.scalar.activation(std[:], vs[:], AF.Sqrt, bias=eps_ap, scale=1.0 / DM)
        rs = stats.tile([128, 1], F32, tag="rs", bufs=3, name="rs")
        nc.vector.reciprocal(rs[:], std[:])
        xn = pool.tile([128, DM], F32, tag="ln_xn", bufs=2, name="ln_xn")
        nc.vector.tensor_scalar(xn[:], xc[:], rs[:], None, ALU.mult)
        if ident_norm:
            return xn
        nc.vector.tensor_tensor(xc[:], xn[:], g_bc[:], ALU.mult)
        lnout = pool.tile([128, DM], F32, tag="ln_out", bufs=2, name="ln_out")
        nc.vector.tensor_tensor(lnout[:], xc[:], b_bc[:], ALU.add)
        return lnout

    def pe_transpose_tile(pspool, src_b16, dstT, r, ident_b):
        """Transpose [128, DM] bf16 -> dstT[:, dj, r] for dj in 0..ND-1 via PE."""
        for g in range(2):
            ptr = pspool.tile([128, 512], BF16, tag="tr", bufs=2, name="ptr")
            for k in range(4):
                dj = g * 4 + k
                nc.tensor.matmul(ptr[:, k * 128:(k + 1) * 128],
                                 src_b16[:, dj * 128:(dj + 1) * 128], ident_b[:],
                                 is_transpose=True, start=True, stop=True)
            nc.scalar.activation(dstT[:, g * 4:(g + 1) * 4, r], ptr[:],
                                 AF.Copy)

    with tile.TileContext(nc) as tc:
        with tc.tile_pool(name="dram", bufs=1, space="DRAM") as dram, \
             tc.tile_pool(name="const", bufs=1) as constp, \
             tc.tile_pool(name="keep", bufs=1) as keep, \
             tc.tile_pool(name="stats", bufs=1) as stats:

            # ---------------- DRAM scratch ----------------
            def dt_(shape, dtype, nm, shared=False):
                return dram.tile(shape, dtype, tag=nm, name=nm,
                                 addr_space="Shared" if shared else "Local")
            qp_in = dt_([B, DM], F32, "qp_in")
            qp_out = dt_([B, DM], F32, "qp_out", True)
            om_in = [dt_([B, RS], F32, f"om_in{k}") for k in range(2)]
            om_out = [dt_([NCORES, B, RS], F32, f"om_out{k}", True)
                      for k in range(2)]
            c_in = dt_([DS, DM + 1], F32, "c_in")
            c_out = dt_([DS, DM + 1], F32, "c_out")
            qn1_sp = dt_([T, DM], BF16, "qn1_sp")
            qint_sp = dt_([T, DM], BF16, "qint_sp")
            acc_sp = dt_([T, DM], BF16, "acc_sp")
            halo_in = [dt_([128, 2 * NIH], BF16, f"halo_in{k}") for k in range(2)]
            halo_out = [dt_([2, 128, 2 * NIH], BF16, f"halo_out{k}")
                        for k in range(2)]

            # ---------------- constants ----------------
            ones_b = constp.tile([128, 1], BF16, tag="ones_b", name="ones_b")
            nc.vector.memset(ones_b[:], 1.0)
            eps_c = constp.tile([128, 1], F32, tag="eps_c", name="eps_c")
            nc.vector.memset(eps_c[:], 1e-5)
            ident = constp.tile([128, 128], F32, tag="ident", name="ident")
            make_identity(nc, ident)
            ident_b = constp.tile([128, 128], BF16, tag="ident_b", name="ident_b")
            make_identity(nc, ident_b)
            bq = constp.tile([DS, 2], F32, tag="bq", name="bq")
            nc.sync.dma_start(bq[:], bias_q[:])
            bk_bc = constp.tile([128, DS], F32, tag="bk_bc", name="bk_bc")
            nc.sync.dma_start(bk_bc[:], bias_k_bc[:])
            dwk_sb = constp.tile([128, 2 * NIH, 3], F32, tag="dwk", name="dwk_sb")
            nc.sync.dma_start(dwk_sb[:], dwk[:])
            dwk_b = constp.tile([128, 2 * NIH, 3], BF16, tag="dwk_b", name="dwk_b")
            nc.gpsimd.dma_start(dwk_b[:], dwk[:])
            bsel_sb = constp.tile([128, B], F32, tag="bsel", name="bsel_sb")
            nc.sync.dma_start(bsel_sb[:], bsel[:])
            lrsel_sb = constp.tile([128, 4], F32, tag="lrsel", name="lrsel_sb")
            nc.sync.dma_start(lrsel_sb[:], lrsel[:])
            if ident_norm:
                n1g_bc = n1b_bc = n2g_bc = n2b_bc = None
            else:
                n1g_bc = keep.tile([128, DM], F32, tag="n1g", name="n1g_bc")
                nc.sync.dma_start(n1g_bc[:], nrm_bc[0])
                n1b_bc = keep.tile([128, DM], F32, tag="n1b", name="n1b_bc")
                nc.sync.dma_start(n1b_bc[:], nrm_bc[1])
                n2g_bc = keep.tile([128, DM], F32, tag="n2g", name="n2g_bc")
                nc.sync.dma_start(n2g_bc[:], nrm_bc[2])
                n2b_bc = keep.tile([128, DM], F32, tag="n2b", name="n2b_bc")
                nc.sync.dma_start(n2b_bc[:], nrm_bc[3])

            # long-lived across phases
            qn2T = keep.tile([128, ND, T], BF16, tag="qn2T", name="qn2T")
            rN = keep.tile([128, NT], F32, tag="rN", name="rN")

            with tc.tile_pool(name="transp", bufs=1) as transp:
                qrotT = transp.tile([128, ND, T], BF16, tag="tbig", name="qrotT")

                # ============ phase 1: LN1, q_pool (pass A); RoPE (pass B) ============
                with tc.tile_pool(name="w1", bufs=1) as w1, \
                     tc.tile_pool(name="ps1", bufs=1, space="PSUM") as ps1:
                    psqp = [ps1.tile([1, 512], F32, tag="qp", bufs=2, name=f"psqp{k}")
                            for k in range(2)]

                    for i in range(NT):
                        r = slice(i * 128, (i + 1) * 128)
                        qt = w1.tile([128, DM], F32, tag="qt", bufs=2, name="qt")
                        nc.sync.dma_start(qt[:], q_in[r, :])
                        qn1f = ln_tile(w1, stats, qt, n1g_bc, n1b_bc, eps_c[:])
                        qn1b = w1.tile([128, DM], BF16, tag="qn1b", bufs=2,
                                       name="qn1b")
                        nc.scalar.activation(qn1b[:], qn1f[:], AF.Copy)
                        nc.sync.dma_start(qn1_sp[r, :], qn1b[:])
                        for hf in range(2):
                            cs = slice(hf * 512, (hf + 1) * 512)
                            nc.tensor.matmul(psqp[hf][:], ones_b[:], qn1b[:, cs],
                                             start=(i == 0), stop=(i == NT - 1))

                    # ---- q_pool allreduce (fires before RoPE pass) ----
                    qp_stage = w1.tile([1, DM], F32, tag="qp_stage", name="qp_stage")
                    z4 = w1.tile([B, DM], F32, tag="z4", name="z4")
                    for hf in range(2):
                        cs = slice(hf * 512, (hf + 1) * 512)
                        nc.scalar.activation(qp_stage[:, cs], psqp[hf][:], AF.Copy,
                                             scale=1.0 / N)
                        ps4 = ps1.tile([B, 512], F32, tag="b4", bufs=2, name="ps4")
                        nc.tensor.matmul(ps4[:], bsel_sb[0:1, :], qp_stage[:, cs],
                                         start=True, stop=True)
                        nc.scalar.activation(z4[:, cs], ps4[:], AF.Copy)
                    nc.gpsimd.dma_start(qp_in[:], z4[:])
                    nc.gpsimd.collective_compute(
                        "AllReduce", ALU.add, replica_groups=GRP_ALL,
                        ins=[qp_in.opt()], outs=[qp_out.opt()])

                    # ---- pass B: RoPE in bf16 (overlaps AR + hyper stream) ----
                    for i in range(NT):
                        r = slice(i * 128, (i + 1) * 128)
                        qnb = w1.tile([128, DM], BF16, tag="qnb", bufs=2,
                                      name="qnb")
                        nc.sync.dma_start(qnb[:], qn1_sp[r, :])
                        ctb = w1.tile([128, DM], BF16, tag="ctb", bufs=2,
                                      name="ctb")
                        nc.gpsimd.dma_start(ctb[:], cosn[r, :])
                        stb = w1.tile([128, DM], BF16, tag="stb", bufs=2,
                                      name="stb")
                        nc.gpsimd.dma_start(stb[:], sinn[r, :])
                        t1 = w1.tile([128, DM], BF16, tag="t1", bufs=2, name="t1")
                        nc.vector.tensor_tensor(t1[:], qnb[:], ctb[:], ALU.mult)
                        u1 = w1.tile([128, 512], BF16, tag="u1", bufs=2, name="u1")
                        nc.vector.tensor_tensor(u1[:], qnb[:, 512:], stb[:, :512],
                                                ALU.mult)
                        qr = w1.tile([128, DM], BF16, tag="qr", bufs=2, name="qr")
                        nc.vector.tensor_tensor(qr[:, :512], t1[:, :512], u1[:],
                                                ALU.subtract)
                        u2 = w1.tile([128, 512], BF16, tag="u2", bufs=2, name="u2")
                        nc.vector.tensor_tensor(u2[:], qnb[:, :512], stb[:, 512:],
                                                ALU.mult)
                        nc.vector.tensor_tensor(qr[:, 512:], t1[:, 512:], u2[:],
                                                ALU.add)
                        pe_transpose_tile(ps1, qr, qrotT, r, ident_b)

                # ============ phase 2: hyper GEMM + Omega allgather ============
                with tc.tile_pool(name="w2", bufs=1) as w2, \
                     tc.tile_pool(name="ps2", bufs=1, space="PSUM") as ps2:
                    qp4 = w2.tile([B, DM], F32, tag="qp4", name="qp4")
                    nc.gpsimd.dma_start(qp4[:], qp_out[:])
                    qpT = w2.tile([128, ND, B], BF16, tag="qpT", name="qpT")
                    for dj in range(ND):
                        pst = ps2.tile([128, B], F32, tag="tp", bufs=2, name="pst")
                        nc.tensor.transpose(pst[:], qp4[:, dj * 128:(dj + 1) * 128],
                                            ident[0:B, 0:B])
                        nc.scalar.activation(qpT[:, dj, :], pst[:], AF.Copy)

                    for mat in (1, 0):  # K first: Phi_K consumers unblock sooner
                        for rc in range(RS // 512):
                            rcs = slice(rc * 512, (rc + 1) * 512)
                            hk = w2.tile([128, ND, 512], BF16, tag="hk", bufs=4,
                                         name="hk")
                            nc.sync.dma_start(
                                hk[:],
                                hqk[mat, :, rcs].rearrange("(dj p) r -> p dj r",
                                                           p=128))
                            pso = ps2.tile([B, 512], F32, tag="b4", bufs=4,
                                           name="pso")
                            for dj in range(ND):
                                nc.tensor.matmul(pso[:], qpT[:, dj, :], hk[:, dj, :],
                                                 start=(dj == 0),
                                                 stop=(dj == ND - 1))
                            st512 = w2.tile([B, 512], F32, tag="st512", bufs=4,
                                            name="st512")
                            nc.vector.tensor_copy(st512[:], pso[:])
                            nc.sync.dma_start(om_in[mat][:, rcs], st512[:])
                        nc.gpsimd.collective_compute(
                            "AllGather", ALU.bypass, replica_groups=GRP_ALL,
                            ins=[om_in[mat].opt()], outs=[om_out[mat].opt()])

                # ============ phases 3-4 ============
                with tc.tile_pool(name="k35", bufs=1) as k35:
                    om_sb = k35.tile([128, 2, ND, DS], BF16, tag="om_sb",
                                     name="om_sb")
                    phiK = k35.tile([128, NT, DS], BF16, tag="phiK", name="phiK")
                    phiQT = k35.tile([DS, NT, 128], BF16, tag="phiQT", name="phiQT")
                    c_b = k35.tile([DS, DM], BF16, tag="c_b", name="c_b")
                    pks_b = k35.tile([DS, 1], BF16, tag="pks_b", name="pks_b")
                    mprojb = k35.tile([128, ND, DM], BF16, tag="mprojb",
                                      name="mprojb")
                    nc.gpsimd.dma_start(
                        mprojb[:],
                        mprojT[:, :].rearrange("(dj p) n -> p dj n", p=128))

                    with tc.tile_pool(name="w34", bufs=1) as w34, \
                         tc.tile_pool(name="ps3", bufs=1, space="PSUM") as ps3:
                        for mat in (1, 0):
                            for dj in range(ND):
                                for b in range(B):
                                    obt = w34.tile([128, DS], BF16, tag="obt",
                                                   bufs=4, name="obt")
                                    nc.gpsimd.dma_start(
                                        obt[:],
                                        om_out[mat][dj, b].rearrange(
                                            "(p e) -> p e", p=128))
                                    if b == 0:
                                        nc.vector.tensor_scalar(
                                            om_sb[:, mat, dj, :], obt[:],
                                            bsel_sb[:, 0:1], None, ALU.mult)
                                    else:
                                        nc.vector.scalar_tensor_tensor(
                                            om_sb[:, mat, dj, :], obt[:],
                                            bsel_sb[:, b:b + 1],
                                            om_sb[:, mat, dj, :],
                                            ALU.mult, ALU.add)

                        for i in range(NT):
                            r = slice(i * 128, (i + 1) * 128)
                            pk = ps3.tile([128, DS], F32, tag="phi", bufs=2,
                                          name="pk")
                            for dj in range(ND):
                                nc.tensor.matmul(pk[:], qrotT[:, dj, r],
                                                 om_sb[:, 1, dj, :],
                                                 start=(dj == 0),
                                                 stop=(dj == ND - 1))
                            zf = w34.tile([128, DS], F32, tag="zf", bufs=2,
                                          name="zf")
                            nc.vector.tensor_tensor(zf[:], pk[:], bk_bc[:], ALU.add)
                            rf = w34.tile([128, DS], F32, tag="rf", bufs=2,
                                          name="rf")
                            nc.vector.tensor_scalar(rf[:], zf[:], 0.0, None,
                                                    ALU.max)
                            ef = w34.tile([128, DS], F32, tag="ef", bufs=2,
                                          name="ef")
                            nc.vector.tensor_tensor(ef[:], zf[:], rf[:],
                                                    ALU.subtract)
                            nc.scalar.activation(ef[:], ef[:], AF.Exp)
                            nc.vector.tensor_tensor(phiK[:, i, :], rf[:], ef[:],
                                                    ALU.add)

                            pq = ps3.tile([DS, 128], F32, tag="phiq", bufs=2,
                                          name="pq")
                            for dj in range(ND):
                                nc.tensor.matmul(pq[:], om_sb[:, 0, dj, :],
                                                 qrotT[:, dj, r],
                                                 start=(dj == 0),
                                                 stop=(dj == ND - 1))
                            zq = w34.tile([DS, 128], F32, tag="zq", bufs=2,
                                          name="zq")
                            nc.vector.tensor_scalar(zq[:], pq[:], bq[:, 0:1], None,
                                                    ALU.add)
                            rq = w34.tile([DS, 128], F32, tag="rq", bufs=2,
                                          name="rq")
                            nc.vector.tensor_scalar(rq[:], zq[:], 0.0, None,
                                                    ALU.max)
                            eq = w34.tile([DS, 128], F32, tag="eq", bufs=2,
                                          name="eq")
                            nc.vector.tensor_tensor(eq[:], zq[:], rq[:],
                                                    ALU.subtract)
                            nc.scalar.activation(eq[:], eq[:], AF.Exp)
                            nc.vector.tensor_tensor(phiQT[:, i, :], rq[:], eq[:],
                                                    ALU.add)

                        # ---- C GEMM + phi_k_sum + pair allreduce ----
                        psc = [ps3.tile([DS, 512], F32, tag="c", bufs=2,
                                        name=f"psc{k}") for k in range(2)]
                        psk = ps3.tile([DS, 1], F32, tag="pks", bufs=1, name="psk")
                        for i in range(NT):
                            qn1t = w34.tile([128, DM], BF16, tag="qn1t", bufs=3,
                                            name="qn1t")
                            nc.sync.dma_start(qn1t[:],
                                              qn1_sp[i * 128:(i + 1) * 128, :])
                            for nb in range(2):
                                nc.tensor.matmul(psc[nb][:], phiK[:, i, :],
                                                 qn1t[:, nb * 512:(nb + 1) * 512],
                                                 start=(i == 0), stop=(i == NT - 1))
                            nc.tensor.matmul(psk[:], phiK[:, i, :], ones_b[:],
                                             start=(i == 0), stop=(i == NT - 1))
                        c_stage = w34.tile([DS, DM + 1], F32, tag="c_stage",
                                           name="c_stage")
                        for nb in range(2):
                            nc.vector.tensor_copy(
                                c_stage[:, nb * 512:(nb + 1) * 512], psc[nb][:])
                        nc.vector.tensor_copy(c_stage[:, DM:DM + 1], psk[:])
                        nc.gpsimd.dma_start(c_in[:], c_stage[:])
                        nc.gpsimd.collective_compute(
                            "AllReduce", ALU.add, replica_groups=GRP_PAIR,
                            ins=[c_in.opt()], outs=[c_out.opt()])

                        nc.gpsimd.dma_start(c_b[:], c_out[:, 0:DM])
                        nc.gpsimd.dma_start(pks_b[:], c_out[:, DM:DM + 1])

                    # ============ phase 5 ============
                    mT = transp.tile([128, ND, T], BF16, tag="tbig", name="mT")
                    with tc.tile_pool(name="w5", bufs=1) as w5, \
                         tc.tile_pool(name="ps5", bufs=1, space="PSUM") as ps5:
                        for i in range(NT):
                            r = slice(i * 128, (i + 1) * 128)
                            pd = ps5.tile([128, 1], F32, tag="d1", bufs=2, name="pd")
                            nc.tensor.matmul(pd[:], phiQT[:, i, :], pks_b[:],
                                             start=True, stop=True)
                            ab = stats.tile([128, 1], F32, tag="ab", bufs=2,
                                            name="ab")
                            nc.scalar.activation(ab[:], pd[:], AF.Abs)
                            nc.vector.tensor_scalar(ab[:], ab[:], 1.0, None,
                                                    ALU.add)
                            nc.vector.reciprocal(rN[:, i:i + 1], ab[:])

                            qn1t = w5.tile([128, DM], F32, tag="qn1tf", bufs=2,
                                           name="qn1tf")
                            nc.gpsimd.dma_start(qn1t[:], qn1_sp[r, :])
                            mb = w5.tile([128, DM], BF16, tag="mb", bufs=2,
                                         name="mb")
                            for nb in range(2):
                                cs = slice(nb * 512, (nb + 1) * 512)
                                pa = ps5.tile([128, 512], F32, tag="mm512", bufs=4,
                                              name="pa")
                                nc.tensor.matmul(pa[:], phiQT[:, i, :], c_b[:, cs],
                                                 start=True, stop=True)
                                nc.vector.scalar_tensor_tensor(
                                    mb[:, cs], pa[:], rN[:, i:i + 1], qn1t[:, cs],
                                    ALU.mult, ALU.subtract)
                            pe_transpose_tile(ps5, mb, mT, r, ident_b)

                            baset = w5.tile([128, DM], F32, tag="baset", bufs=2,
                                            name="baset")
                            nc.sync.dma_start(baset[:], base_in[r, :])
                            qint = w5.tile([128, DM], F32, tag="qint", bufs=1,
                                           name="qint")
                            pms = [ps5.tile([128, 512], F32, tag="mm512", bufs=4,
                                            name=f"pm{k}") for k in range(2)]
                            for dj in range(ND):
                                for nb in range(2):
                                    cs = slice(nb * 512, (nb + 1) * 512)
                                    nc.tensor.matmul(pms[nb][:], mT[:, dj, r],
                                                     mprojb[:, dj, cs],
                                                     start=(dj == 0),
                                                     stop=(dj == ND - 1))
                            for nb in range(2):
                                cs = slice(nb * 512, (nb + 1) * 512)
                                nc.vector.scalar_tensor_tensor(
                                    qint[:, cs], pms[nb][:], dt_safe, baset[:, cs],
                                    ALU.mult, ALU.add)
                            qint_b = w5.tile([128, DM], BF16, tag="qint_b", bufs=2,
                                             name="qint_b")
                            nc.scalar.activation(qint_b[:], qint[:], AF.Copy)
                            nc.sync.dma_start(qint_sp[r, :], qint_b[:])
                            qn2f = ln_tile(w5, stats, qint, n2g_bc, n2b_bc,
                                           eps_c[:])
                            qn2b = w5.tile([128, DM], BF16, tag="qn2b", bufs=2,
                                           name="qn2b")
                            nc.scalar.activation(qn2b[:], qn2f[:], AF.Copy)
                            pe_transpose_tile(ps5, qn2b, qn2T, r, ident_b)

            # ============ phases 6-7: FFN ============
            with tc.tile_pool(name="ffn", bufs=1) as ffn, \
                 tc.tile_pool(name="ws", bufs=1) as ws:
                H = ffn.tile([128, NIH, T], BF16, tag="H", name="H")
                for half in range(2):
                    ibase = half * NIH
                    sv = ws.tile([128, NIH, 4], BF16, tag="sv", name="sv")
                    with tc.tile_pool(name="psgu", bufs=1, space="PSUM") as psgu:
                        for mi in range(NIH):
                            g0 = half * 2048 + mi * 128
                            u0 = INNER + g0
                            wu_g = ws.tile([128, ND, 128], BF16, tag="wu_g", bufs=3,
                                           name="wu_g")
                            nc.gpsimd.dma_start(
                                wu_g[:],
                                wupT[:, g0:g0 + 128].rearrange(
                                    "(dj p) f -> p dj f", p=128))
                            wu_u = ws.tile([128, ND, 128], BF16, tag="wu_u", bufs=3,
                                           name="wu_u")
                            nc.gpsimd.dma_start(
                                wu_u[:],
                                wupT[:, u0:u0 + 128].rearrange(
                                    "(dj p) f -> p dj f", p=128))
                            htmp = ws.tile([128, T], BF16, tag="htmp", bufs=3,
                                           name="htmp")
                            for nb in range(2):
                                cs0 = nb * 1024
                                pg = psgu.tile([128, 1024], F32, tag="pg", bufs=2,
                                               name="pg")
                                pu = psgu.tile([128, 1024], F32, tag="pu", bufs=2,
                                               name="pu")
                                for hb in range(2):
                                    cs = slice(cs0 + hb * 512, cs0 + (hb + 1) * 512)
                                    ps_s = slice(hb * 512, (hb + 1) * 512)
                                    for dj in range(ND):
                                        nc.tensor.matmul(pg[:, ps_s], wu_g[:, dj, :],
                                                         qn2T[:, dj, cs],
                                                         start=(dj == 0),
                                                         stop=(dj == ND - 1))
                                    for dj in range(ND):
                                        nc.tensor.matmul(pu[:, ps_s], wu_u[:, dj, :],
                                                         qn2T[:, dj, cs],
                                                         start=(dj == 0),
                                                         stop=(dj == ND - 1))
                                gt = ws.tile([128, 1024], F32, tag="gt", bufs=2,
                                             name="gt")
                                nc.scalar.activation(gt[:], pg[:], AF.Silu)
                                nc.vector.tensor_tensor(htmp[:, cs0:cs0 + 1024],
                                                        gt[:], pu[:], ALU.mult)
                            # save raw boundary cols for the post-halo patch
                            nc.gpsimd.tensor_copy(sv[:, mi, 0:2], htmp[:, 0:2])
                            nc.gpsimd.tensor_copy(sv[:, mi, 2:4], htmp[:, T - 2:T])
                            # conv tokens 1..T-2 directly into H (edges patched)
                            w0 = dwk_sb[:, ibase + mi, 0:1]
                            wm = dwk_sb[:, ibase + mi, 1:2]
                            w2_ = dwk_sb[:, ibase + mi, 2:3]
                            hsl = H[:, mi, 1:T - 1]
                            nc.vector.tensor_scalar(hsl, htmp[:, 0:T - 2], w0,
                                                    None, ALU.mult)
                            nc.vector.scalar_tensor_tensor(hsl, htmp[:, 1:T - 1],
                                                           wm, hsl, ALU.mult,
                                                           ALU.add)
                            nc.vector.scalar_tensor_tensor(hsl, htmp[:, 2:T],
                                                           w2_, hsl, ALU.mult,
                                                           ALU.add)
                    # ---- halo exchange + boundary patch ----
                    hstage = ws.tile([128, 2 * NIH], BF16, tag="hstage",
                                     name="hstage")
                    nc.vector.tensor_copy(hstage[:, 0:NIH], sv[:, :, 0])
                    nc.vector.tensor_copy(hstage[:, NIH:2 * NIH], sv[:, :, 3])
                    nc.gpsimd.dma_start(halo_in[half][:], hstage[:])
                    nc.gpsimd.collective_compute(
                        "AllGather", ALU.bypass, replica_groups=GRP_PAIR,
                        ins=[halo_in[half].opt()], outs=[halo_out[half].opt()])
                    hg0 = ws.tile([128, 2 * NIH], BF16, tag="hg0", name="hg0")
                    nc.gpsimd.dma_start(hg0[:], halo_out[half][0])
                    hg1 = ws.tile([128, 2 * NIH], BF16, tag="hg1", name="hg1")
                    nc.gpsimd.dma_start(hg1[:], halo_out[half][1])
                    halL = ws.tile([128, NIH], BF16, tag="halL", name="halL")
                    nc.vector.tensor_scalar(halL[:], hg0[:, NIH:2 * NIH],
                                            lrsel_sb[:, 0:1], None, ALU.mult)
                    nc.vector.scalar_tensor_tensor(halL[:], hg1[:, NIH:2 * NIH],
                                                   lrsel_sb[:, 1:2], halL[:],
                                                   ALU.mult, ALU.add)
                    halR = ws.tile([128, NIH], BF16, tag="halR", name="halR")
                    nc.vector.tensor_scalar(halR[:], hg0[:, 0:NIH],
                                            lrsel_sb[:, 2:3], None, ALU.mult)
                    nc.vector.scalar_tensor_tensor(halR[:], hg1[:, 0:NIH],
                                                   lrsel_sb[:, 3:4], halR[:],
                                                   ALU.mult, ALU.add)
                    w0v = dwk_b[:, ibase:ibase + NIH, 0]
                    w1v = dwk_b[:, ibase:ibase + NIH, 1]
                    w2v = dwk_b[:, ibase:ibase + NIH, 2]
                    pt1 = ws.tile([128, NIH], BF16, tag="pt1", name="pt1")
                    pt2 = ws.tile([128, NIH], BF16, tag="pt2", name="pt2")
                    nc.vector.tensor_tensor(pt1[:], halL[:], w0v, ALU.mult)
                    nc.vector.tensor_tensor(pt2[:], sv[:, :, 0], w1v, ALU.mult)
                    nc.vector.tensor_tensor(pt1[:], pt1[:], pt2[:], ALU.add)
                    nc.vector.tensor_tensor(pt2[:], sv[:, :, 1], w2v, ALU.mult)
                    nc.vector.tensor_tensor(H[:, :, 0], pt1[:], pt2[:], ALU.add)
                    nc.vector.tensor_tensor(pt1[:], sv[:, :, 2], w0v, ALU.mult)
                    nc.vector.tensor_tensor(pt2[:], sv[:, :, 3], w1v, ALU.mult)
                    nc.vector.tensor_tensor(pt1[:], pt1[:], pt2[:], ALU.add)
                    nc.vector.tensor_tensor(pt2[:], halR[:], w2v, ALU.mult)
                    nc.vector.tensor_tensor(H[:, :, T - 1], pt1[:], pt2[:],
                                            ALU.add)
                    # ---- W_down GEMM (dm in 2 halves of 512) ----
                    with tc.tile_pool(name="pswd", bufs=1, space="PSUM") as pswd:
                        for dmq in range(2):
                            ns = slice(dmq * 512, (dmq + 1) * 512)
                            wd = ws.tile([128, NIH, 512], BF16, tag="wd", bufs=2,
                                         name="wd")
                            nc.gpsimd.dma_start(
                                wd[:],
                                wdownT[half * 2048:(half + 1) * 2048, ns]
                                .rearrange("(mi p) n -> p mi n", p=128))
                            for mt in range(NT):
                                ms = slice(mt * 128, (mt + 1) * 128)
                                rr = slice(mt * 128, (mt + 1) * 128)
                                po = pswd.tile([128, 512], F32, tag="wdp", bufs=4,
                                               name="po")
                                for mi in range(NIH):
                                    nc.tensor.matmul(po[:], H[:, mi, ms],
                                                     wd[:, mi, :],
                                                     start=(mi == 0),
                                                     stop=(mi == NIH - 1))
                                if half == 0:
                                    qiv = ws.tile([128, 512], BF16, tag="qiv",
                                                  bufs=2, name="qiv")
                                    nc.sync.dma_start(qiv[:], qint_sp[rr, ns])
                                    qif = ws.tile([128, 512], F32, tag="qif",
                                                  bufs=2, name="qif")
                                    nc.scalar.activation(qif[:], qiv[:], AF.Copy)
                                    a0 = ws.tile([128, 512], BF16, tag="a0",
                                                 bufs=2, name="a0")
                                    nc.vector.tensor_tensor(a0[:], po[:], qif[:],
                                                            ALU.add)
                                    nc.sync.dma_start(acc_sp[rr, ns], a0[:])
                                else:
                                    av = ws.tile([128, 512], BF16, tag="av",
                                                 bufs=2, name="av")
                                    nc.sync.dma_start(av[:], acc_sp[rr, ns])
                                    af = ws.tile([128, 512], F32, tag="af",
                                                 bufs=2, name="af")
                                    nc.scalar.activation(af[:], av[:], AF.Copy)
                                    ot = ws.tile([128, 512], F32, tag="ot",
                                                 bufs=2, name="ot")
                                    nc.vector.tensor_tensor(ot[:], po[:], af[:],
                                                            ALU.add)
                                    nc.sync.dma_start(out_ext[rr, ns], ot[:])

    nc.compile()
    return nc


def _build_cached(dt_safe, lam_safe, ident_norm):
    key = (round(float(dt_safe), 8), round(float(lam_safe), 8), bool(ident_norm))
    if key not in _cache:
        _cache[key] = build(float(dt_safe), float(lam_safe), bool(ident_norm))
    return _cache[key]


def kernel(**inputs):
    _install_ntff_shim()
    Q_in = np.ascontiguousarray(inputs["Q_in"], dtype=np.float32)
    X = np.ascontiguousarray(inputs["X"], dtype=np.float32)
    cos = np.ascontiguousarray(inputs["cos"], dtype=np.float32)
    sin = np.ascontiguousarray(inputs["sin"], dtype=np.float32)
    hyper_q_w = np.asarray(inputs["hyper_q_w"], dtype=np.float32)
    hyper_k_w = np.asarray(inputs["hyper_k_w"], dtype=np.float32)
    B_Q = np.asarray(inputs["B_Q"], dtype=np.float32)
    B_K = np.asarray(inputs["B_K"], dtype=np.float32)
    W_up = np.asarray(inputs["W_up"], dtype=np.float32)
    dw = np.asarray(inputs["dw_conv_w"], dtype=np.float32)
    W_down = np.asarray(inputs["W_down"], dtype=np.float32)
    m_proj_w = np.asarray(inputs["m_proj_w"], dtype=np.float32)
    m_proj_b = np.asarray(inputs["m_proj_b"], dtype=np.float32)
    n1g = np.asarray(inputs["norm1_g"], dtype=np.float32)
    n1b = np.asarray(inputs["norm1_b"], dtype=np.float32)
    n2g = np.asarray(inputs["norm2_g"], dtype=np.float32)
    n2b = np.asarray(inputs["norm2_b"], dtype=np.float32)
    dt = float(np.asarray(inputs["dt"]))
    lam = float(np.asarray(inputs["lam"]))

    dt_safe = float(np.log1p(np.exp(dt)))
    lam_safe = float(np.log1p(np.exp(lam)))
    ident_norm = bool(np.all(n1g == 1) and np.all(n1b == 0)
                      and np.all(n2g == 1) and np.all(n2b == 0)
                      and np.all(m_proj_b == 0))

    nc = _build_cached(dt_safe, lam_safe, ident_norm)

    # host-side sharding / layout prep
    hyperT = np.stack([hyper_q_w.T, hyper_k_w.T]).astype(ml_dtypes.bfloat16)
    wupT = np.ascontiguousarray(W_up.T)
    wdownT = np.ascontiguousarray(W_down.T)
    mprojT = np.ascontiguousarray(m_proj_w.T)
    nrm_bc = np.stack([
        np.broadcast_to(n1g, (128, DM)),
        np.broadcast_to(n1b, (128, DM)),
        np.broadcast_to(n2g, (128, DM)),
        np.broadcast_to(n2b, (128, DM)),
    ]).astype(np.float32)
    bias_q = np.stack([B_Q, -B_Q], axis=1).astype(np.float32)
    bias_k_bc = np.ascontiguousarray(np.broadcast_to(B_K, (128, DS)),
                                     dtype=np.float32)
    dwk = np.ascontiguousarray(
        dw[:, 0, :].reshape(2 * NIH, 128, 3).transpose(1, 0, 2),
        dtype=np.float32)
    base_full = ((1.0 - lam_safe) * Q_in + lam_safe * X
                 + (dt_safe * m_proj_b)[None, None, :]).astype(np.float32)

    in_maps = []
    for c in range(NCORES):
        b, h = c // 2, c % 2
        tok = slice(h * T, (h + 1) * T)
        rsl = slice(c * RS, (c + 1) * RS)
        bsel = np.zeros((128, B), np.float32)
        bsel[:, b] = 1.0
        lrsel = np.zeros((128, 4), np.float32)
        if h == 0:
            lrsel[:, 3] = 1.0
        else:
            lrsel[:, 0] = 1.0
        in_maps.append({
            "q_in": np.ascontiguousarray(Q_in[b, tok]),
            "base_in": np.ascontiguousarray(base_full[b, tok]),
            "cosn": np.ascontiguousarray(cos[tok]),
            "sinn": np.ascontiguousarray(sin[tok]),
            "hqk": np.ascontiguousarray(hyperT[:, :, rsl]),
            "wupT": wupT,
            "wdownT": wdownT,
            "mprojT": mprojT,
            "nrm_bc": nrm_bc,
            "bias_q": bias_q,
            "bias_k_bc": bias_k_bc,
            "dwk": dwk,
            "bsel": bsel,
            "lrsel": lrsel,
        })

    trace = bool(os.environ.get("BASS_KERNEL_TRACE"))
    res = run_bass_kernel_spmd(nc, in_maps, core_ids=list(range(NCORES)),
                               trace=trace,
                               tmpdir=os.environ.get("BASS_KERNEL_TMPDIR"))
    kernel._last = res

    out = np.empty((B, N, DM), np.float32)
    for c in range(NCORES):
        b, h = c // 2, c % 2
        out[b, h * T:(h + 1) * T] = res.results[c]["out"]
    return out


# revision 15
# speedup vs baseline: 1.9608x; 1.2968x over previous
"""Trainium2 Bass kernel for nn_AMK_Block (sparse_attention), 8 NeuronCores.

Sharding: core c => (batch b = c//2, seq half h = c%2), T=2048 tokens/core.
Collectives: q_pool AllReduce (8 cores), Omega AllGather (8, hyper GEMM
row-sharded), C/phi_k_sum pair AllReduce, conv-halo pair AllGather.
Heavy GEMMs in bf16 (fp32 accumulate); norms/elementwise mostly fp32.
This build specializes for identity norm affine (g==1, b==0) and
m_proj_b==0; kernel() verifies and falls back to a general build.
"""
import os
import sys
import types
import numpy as np
import ml_dtypes

import concourse.bass as bass
import concourse.mybir as mybir
import concourse.tile as tile
from concourse import bacc
from concourse.bass_utils import run_bass_kernel_spmd
from concourse.masks import make_identity

F32 = mybir.dt.float32
BF16 = mybir.dt.bfloat16
FP8 = mybir.dt.float8e4
DR = mybir.MatmulPerfMode.DoubleRow
AF = mybir.ActivationFunctionType
ALU = mybir.AluOpType
AX = mybir.AxisListType

NCORES = 8
B, N, DM, DS = 4, 4096, 1024, 64
INNER = 4 * DM
T = N // 2               # tokens per core
NT = T // 128            # 16 tok tiles
ND = DM // 128           # 8 d tiles
NIH = INNER // 128 // 2  # 16 i-tiles per inner half
RS = DM * DS // NCORES   # 8192 hyper rows per core
TPAD = T                 # H holds conv output, no halo pad needed

_cache = {}


def _install_ntff_shim():
    if "antenv.axon_hooks" in sys.modules:
        return
    try:
        from trn_agent_boot.trn_boot import _ntff_profile_via_ctypes
        hook = _ntff_profile_via_ctypes("/opt/axon/libaxon_pjrt.so")
    except Exception:
        hook = None
    m = types.ModuleType("antenv.axon_hooks")
    m.get_axon_ntff_profile_hook = lambda: hook
    m.set_axon_ntff_profile_hook = lambda h: None
    sys.modules["antenv.axon_hooks"] = m


def build(dt_safe: float, lam_safe: float, ident_norm: bool):
    """ident_norm=True assumes norm g==1/b==0 and m_proj_b==0 (host-checked)."""
    nc = bacc.Bacc(None, target_bir_lowering=False, debug=False)

    q_in = nc.dram_tensor("q_in", [T, DM], F32, kind="ExternalInput")
    base_in = nc.dram_tensor("base_in", [T, DM], F32, kind="ExternalInput")
    cosn = nc.dram_tensor("cosn", [T, DM], F32, kind="ExternalInput")
    sinn = nc.dram_tensor("sinn", [T, DM], F32, kind="ExternalInput")
    hqk = nc.dram_tensor("hqk", [2, DM, RS], BF16, kind="ExternalInput")
    wupT = nc.dram_tensor("wupT", [DM, 2 * INNER], FP8, kind="ExternalInput")
    wdownT = nc.dram_tensor("wdownT", [INNER, DM], FP8, kind="ExternalInput")
    mprojT = nc.dram_tensor("mprojT", [DM, DM], F32, kind="ExternalInput")
    nrm_bc = nc.dram_tensor("nrm_bc", [4, 128, DM], F32, kind="ExternalInput")
    bias_q = nc.dram_tensor("bias_q", [DS, 2], F32, kind="ExternalInput")
    bias_k_bc = nc.dram_tensor("bias_k_bc", [128, DS], F32, kind="ExternalInput")
    dwk = nc.dram_tensor("dwk", [128, 2 * NIH, 3], F32, kind="ExternalInput")
    bsel = nc.dram_tensor("bsel", [128, B], F32, kind="ExternalInput")
    lrsel = nc.dram_tensor("lrsel", [128, 4], F32, kind="ExternalInput")
    out_ext = nc.dram_tensor("out", [T, DM], F32, kind="ExternalOutput")

    GRP_ALL = [list(range(NCORES))]
    GRP_PAIR = [[2 * i, 2 * i + 1] for i in range(B)]

    def ln_tile(pool, stats, xf, g_bc, b_bc, eps_ap, sqb=1, out_dtype=None,
                out_tile=None):
        """LayerNorm over free dim of xf [128, DM] f32."""
        smu = stats.tile([128, 1], F32, tag="smu", bufs=3, name="smu")
        nc.vector.tensor_reduce(smu[:], xf[:], AX.X, ALU.add)
        negmu = stats.tile([128, 1], F32, tag="negmu", bufs=3, name="negmu")
        nc.vector.tensor_scalar(negmu[:], smu[:], -1.0 / DM, None, ALU.mult)
        xc = pool.tile([128, DM], F32, tag="ln_xc", bufs=2, name="ln_xc")
        nc.vector.tensor_scalar(xc[:], xf[:], negmu[:], None, ALU.add)
        sq = pool.tile([128, DM], F32, tag="ln_sq", bufs=sqb, name="ln_sq")
        vs = stats.tile([128, 1], F32, tag="vs", bufs=3, name="vs")
        nc.scalar.activation(sq[:], xc[:], AF.Square, accum_out=vs[:])
        std = stats.tile([128, 1], F32, tag="std", bufs=3, name="std")
        nc.scalar.activation(std[:], vs[:], AF.Sqrt, bias=eps_ap, scale=1.0 / DM)
        rs = stats.tile([128, 1], F32, tag="rs", bufs=3, name="rs")
        nc.vector.reciprocal(rs[:], std[:])
        if out_tile is not None and ident_norm:
            nc.vector.tensor_scalar(out_tile[:], xc[:], rs[:], None, ALU.mult)
            return out_tile
        xn = pool.tile([128, DM], F32, tag="ln_xn", bufs=2, name="ln_xn")
        nc.vector.tensor_scalar(xn[:], xc[:], rs[:], None, ALU.mult)
        if ident_norm:
            return xn
        nc.vector.tensor_tensor(xc[:], xn[:], g_bc[:], ALU.mult)
        lnout = pool.tile([128, DM], F32, tag="ln_out", bufs=2, name="ln_out")
        nc.vector.tensor_tensor(lnout[:], xc[:], b_bc[:], ALU.add)
        return lnout

    def pe_transpose_tile(pspool, src_b16, dstT, r, ident_b):
        """Transpose [128, DM] bf16 -> dstT[:, dj, r] for dj in 0..ND-1 via PE."""
        for g in range(2):
            ptr = pspool.tile([128, 512], BF16, tag="tr", bufs=2, name="ptr")
            for k in range(4):
                dj = g * 4 + k
                nc.tensor.matmul(ptr[:, k * 128:(k + 1) * 128],
                                 src_b16[:, dj * 128:(dj + 1) * 128], ident_b[:],
                                 is_transpose=True, start=True, stop=True)
            nc.scalar.activation(dstT[:, g * 4:(g + 1) * 4, r], ptr[:],
                                 AF.Copy)

    with tile.TileContext(nc) as tc:
        with tc.tile_pool(name="dram", bufs=1, space="DRAM") as dram, \
             tc.tile_pool(name="const", bufs=1) as constp, \
             tc.tile_pool(name="keep", bufs=1) as keep, \
             tc.tile_pool(name="stats", bufs=1) as stats:

            # ---------------- DRAM scratch ----------------
            def dt_(shape, dtype, nm, shared=False):
                return dram.tile(shape, dtype, tag=nm, name=nm,
                                 addr_space="Shared" if shared else "Local")
            qp_in = dt_([B, DM], F32, "qp_in")
            qp_out = dt_([B, DM], F32, "qp_out", True)
            om_in = [dt_([B, RS], F32, f"om_in{k}") for k in range(2)]
            om_out = [dt_([NCORES, B, RS], F32, f"om_out{k}", True)
                      for k in range(2)]
            c_in = dt_([DS, DM + 1], F32, "c_in")
            c_out = dt_([DS, DM + 1], F32, "c_out")
            qn1_sp = dt_([T, DM], BF16, "qn1_sp")
            qint_sp = dt_([T, DM], BF16, "qint_sp")
            acc_sp = dt_([T, DM], BF16, "acc_sp")
            halo_in = [dt_([128, 2 * NIH], BF16, f"halo_in{k}") for k in range(2)]
            halo_out = [dt_([2, 128, 2 * NIH], BF16, f"halo_out{k}")
                        for k in range(2)]

            # ---------------- constants ----------------
            ones_b = constp.tile([128, 1], BF16, tag="ones_b", name="ones_b")
            nc.vector.memset(ones_b[:], 1.0)
            eps_c = constp.tile([128, 1], F32, tag="eps_c", name="eps_c")
            nc.vector.memset(eps_c[:], 1e-5)
            ident = constp.tile([128, 128], F32, tag="ident", name="ident")
            make_identity(nc, ident)
            ident_b = constp.tile([128, 128], BF16, tag="ident_b", name="ident_b")
            make_identity(nc, ident_b)
            bq = constp.tile([DS, 2], F32, tag="bq", name="bq")
            nc.sync.dma_start(bq[:], bias_q[:])
            bk_bc = constp.tile([128, DS], F32, tag="bk_bc", name="bk_bc")
            nc.sync.dma_start(bk_bc[:], bias_k_bc[:])
            dwk_sb = constp.tile([128, 2 * NIH, 3], F32, tag="dwk", name="dwk_sb")
            nc.sync.dma_start(dwk_sb[:], dwk[:])
            dwk_b = constp.tile([128, 2 * NIH, 3], BF16, tag="dwk_b", name="dwk_b")
            nc.gpsimd.dma_start(dwk_b[:], dwk[:])
            bsel_sb = constp.tile([128, B], F32, tag="bsel", name="bsel_sb")
            nc.sync.dma_start(bsel_sb[:], bsel[:])
            lrsel_sb = constp.tile([128, 4], F32, tag="lrsel", name="lrsel_sb")
            nc.sync.dma_start(lrsel_sb[:], lrsel[:])
            if ident_norm:
                n1g_bc = n1b_bc = n2g_bc = n2b_bc = None
            else:
                n1g_bc = keep.tile([128, DM], F32, tag="n1g", name="n1g_bc")
                nc.sync.dma_start(n1g_bc[:], nrm_bc[0])
                n1b_bc = keep.tile([128, DM], F32, tag="n1b", name="n1b_bc")
                nc.sync.dma_start(n1b_bc[:], nrm_bc[1])
                n2g_bc = keep.tile([128, DM], F32, tag="n2g", name="n2g_bc")
                nc.sync.dma_start(n2g_bc[:], nrm_bc[2])
                n2b_bc = keep.tile([128, DM], F32, tag="n2b", name="n2b_bc")
                nc.sync.dma_start(n2b_bc[:], nrm_bc[3])

            # long-lived across phases
            qn2T = keep.tile([128, ND, T], FP8, tag="qn2T", name="qn2T")
            rN = keep.tile([128, NT], F32, tag="rN", name="rN")

            with tc.tile_pool(name="transp", bufs=1) as transp:
                qrotT = transp.tile([128, ND, T], BF16, tag="tbig", name="qrotT")

                # ============ phase 1: LN1, q_pool (pass A); RoPE (pass B) ============
                with tc.tile_pool(name="w1", bufs=1) as w1, \
                     tc.tile_pool(name="ps1", bufs=1, space="PSUM") as ps1:
                    psqp = [ps1.tile([1, 512], F32, tag="qp", bufs=2, name=f"psqp{k}")
                            for k in range(2)]

                    for i in range(NT):
                        r = slice(i * 128, (i + 1) * 128)
                        qt = w1.tile([128, DM], F32, tag="qt", bufs=2, name="qt")
                        nc.sync.dma_start(qt[:], q_in[r, :])
                        qn1b = w1.tile([128, DM], BF16, tag="qn1b", bufs=2,
                                       name="qn1b")
                        ln_tile(w1, stats, qt, n1g_bc, n1b_bc, eps_c[:],
                                out_tile=qn1b)
                        nc.sync.dma_start(qn1_sp[r, :], qn1b[:])
                        for hf in range(2):
                            cs = slice(hf * 512, (hf + 1) * 512)
                            nc.tensor.matmul(psqp[hf][:], ones_b[:], qn1b[:, cs],
                                             start=(i == 0), stop=(i == NT - 1))

                    # ---- q_pool allreduce (fires before RoPE pass) ----
                    qp_stage = w1.tile([1, DM], F32, tag="qp_stage", name="qp_stage")
                    z4 = w1.tile([B, DM], F32, tag="z4", name="z4")
                    for hf in range(2):
                        cs = slice(hf * 512, (hf + 1) * 512)
                        nc.scalar.activation(qp_stage[:, cs], psqp[hf][:], AF.Copy,
                                             scale=1.0 / N)
                        ps4 = ps1.tile([B, 512], F32, tag="b4", bufs=2, name="ps4")
                        nc.tensor.matmul(ps4[:], bsel_sb[0:1, :], qp_stage[:, cs],
                                         start=True, stop=True)
                        nc.scalar.activation(z4[:, cs], ps4[:], AF.Copy)
                    nc.gpsimd.dma_start(qp_in[:], z4[:])
                    nc.gpsimd.collective_compute(
                        "AllReduce", ALU.add, replica_groups=GRP_ALL,
                        ins=[qp_in.opt()], outs=[qp_out.opt()])

                    # ---- pass B: RoPE in bf16 (overlaps AR + hyper stream) ----
                    for i in range(NT):
                        r = slice(i * 128, (i + 1) * 128)
                        qnb = w1.tile([128, DM], BF16, tag="qnb", bufs=2,
                                      name="qnb")
                        nc.sync.dma_start(qnb[:], qn1_sp[r, :])
                        ctb = w1.tile([128, DM], BF16, tag="ctb", bufs=2,
                                      name="ctb")
                        nc.gpsimd.dma_start(ctb[:], cosn[r, :])
                        stb = w1.tile([128, DM], BF16, tag="stb", bufs=2,
                                      name="stb")
                        nc.gpsimd.dma_start(stb[:], sinn[r, :])
                        t1 = w1.tile([128, DM], BF16, tag="t1", bufs=2, name="t1")
                        nc.vector.tensor_tensor(t1[:], qnb[:], ctb[:], ALU.mult)
                        u1 = w1.tile([128, 512], BF16, tag="u1", bufs=2, name="u1")
                        nc.vector.tensor_tensor(u1[:], qnb[:, 512:], stb[:, :512],
                                                ALU.mult)
                        qr = w1.tile([128, DM], BF16, tag="qr", bufs=2, name="qr")
                        nc.vector.tensor_tensor(qr[:, :512], t1[:, :512], u1[:],
                                                ALU.subtract)
                        u2 = w1.tile([128, 512], BF16, tag="u2", bufs=2, name="u2")
                        nc.vector.tensor_tensor(u2[:], qnb[:, :512], stb[:, 512:],
                                                ALU.mult)
                        nc.vector.tensor_tensor(qr[:, 512:], t1[:, 512:], u2[:],
                                                ALU.add)
                        pe_transpose_tile(ps1, qr, qrotT, r, ident_b)

                # ============ phase 2: hyper GEMM + Omega allgather ============
                with tc.tile_pool(name="w2", bufs=1) as w2, \
                     tc.tile_pool(name="ps2", bufs=1, space="PSUM") as ps2:
                    qp4 = w2.tile([B, DM], F32, tag="qp4", name="qp4")
                    nc.gpsimd.dma_start(qp4[:], qp_out[:])
                    qpT = w2.tile([128, ND, B], BF16, tag="qpT", name="qpT")
                    for dj in range(ND):
                        pst = ps2.tile([128, B], F32, tag="tp", bufs=2, name="pst")
                        nc.tensor.transpose(pst[:], qp4[:, dj * 128:(dj + 1) * 128],
                                            ident[0:B, 0:B])
                        nc.scalar.activation(qpT[:, dj, :], pst[:], AF.Copy)

                    for mat in (1, 0):  # K first: Phi_K consumers unblock sooner
                        for rc in range(RS // 512):
                            rcs = slice(rc * 512, (rc + 1) * 512)
                            hk = w2.tile([128, ND, 512], BF16, tag="hk", bufs=8,
                                         name="hk")
                            nc.sync.dma_start(
                                hk[:],
                                hqk[mat, :, rcs].rearrange("(dj p) r -> p dj r",
                                                           p=128))
                            pso = ps2.tile([B, 512], F32, tag="b4", bufs=4,
                                           name="pso")
                            for dj in range(ND):
                                nc.tensor.matmul(pso[:], qpT[:, dj, :], hk[:, dj, :],
                                                 start=(dj == 0),
                                                 stop=(dj == ND - 1))
                            st512 = w2.tile([B, 512], F32, tag="st512", bufs=4,
                                            name="st512")
                            nc.vector.tensor_copy(st512[:], pso[:])
                            nc.sync.dma_start(om_in[mat][:, rcs], st512[:])
                        nc.gpsimd.collective_compute(
                            "AllGather", ALU.bypass, replica_groups=GRP_ALL,
                            ins=[om_in[mat].opt()], outs=[om_out[mat].opt()])

                # ============ phases 3-4 ============
                with tc.tile_pool(name="k35", bufs=1) as k35:
                    om_sb = k35.tile([128, 2, ND, DS], BF16, tag="om_sb",
                                     name="om_sb")
                    phiK = k35.tile([128, NT, DS], BF16, tag="phiK", name="phiK")
                    phiQT = k35.tile([DS, NT, 128], BF16, tag="phiQT", name="phiQT")
                    c_b = k35.tile([DS, DM], BF16, tag="c_b", name="c_b")
                    pks_b = k35.tile([DS, 1], BF16, tag="pks_b", name="pks_b")
                    mprojb = k35.tile([128, ND, DM], BF16, tag="mprojb",
                                      name="mprojb")
                    nc.gpsimd.dma_start(
                        mprojb[:],
                        mprojT[:, :].rearrange("(dj p) n -> p dj n", p=128))

                    with tc.tile_pool(name="w34", bufs=1) as w34, \
                         tc.tile_pool(name="ps3", bufs=1, space="PSUM") as ps3:
                        for mat in (1, 0):
                            for dj in range(ND):
                                for b in range(B):
                                    obt = w34.tile([128, DS], BF16, tag="obt",
                                                   bufs=4, name="obt")
                                    nc.gpsimd.dma_start(
                                        obt[:],
                                        om_out[mat][dj, b].rearrange(
                                            "(p e) -> p e", p=128))
                                    if b == 0:
                                        nc.vector.tensor_scalar(
                                            om_sb[:, mat, dj, :], obt[:],
                                            bsel_sb[:, 0:1], None, ALU.mult)
                                    else:
                                        nc.vector.scalar_tensor_tensor(
                                            om_sb[:, mat, dj, :], obt[:],
                                            bsel_sb[:, b:b + 1],
                                            om_sb[:, mat, dj, :],
                                            ALU.mult, ALU.add)

                        for i in range(NT):
                            r = slice(i * 128, (i + 1) * 128)
                            pk = ps3.tile([128, DS], F32, tag="phi", bufs=2,
                                          name="pk")
                            for dj in range(ND):
                                nc.tensor.matmul(pk[:], qrotT[:, dj, r],
                                                 om_sb[:, 1, dj, :],
                                                 start=(dj == 0),
                                                 stop=(dj == ND - 1))
                            zf = w34.tile([128, DS], F32, tag="zf", bufs=2,
                                          name="zf")
                            nc.vector.tensor_tensor(zf[:], pk[:], bk_bc[:], ALU.add)
                            rf = w34.tile([128, DS], F32, tag="rf", bufs=2,
                                          name="rf")
                            nc.vector.tensor_scalar(rf[:], zf[:], 0.0, None,
                                                    ALU.max)
                            ef = w34.tile([128, DS], F32, tag="ef", bufs=2,
                                          name="ef")
                            nc.vector.tensor_tensor(ef[:], zf[:], rf[:],
                                                    ALU.subtract)
                            nc.scalar.activation(ef[:], ef[:], AF.Exp)
                            nc.vector.tensor_tensor(phiK[:, i, :], rf[:], ef[:],
                                                    ALU.add)

                            pq = ps3.tile([DS, 128], F32, tag="phiq", bufs=2,
                                          name="pq")
                            for dj in range(ND):
                                nc.tensor.matmul(pq[:], om_sb[:, 0, dj, :],
                                                 qrotT[:, dj, r],
                                                 start=(dj == 0),
                                                 stop=(dj == ND - 1))
                            zq = w34.tile([DS, 128], F32, tag="zq", bufs=2,
                                          name="zq")
                            nc.vector.tensor_scalar(zq[:], pq[:], bq[:, 0:1], None,
                                                    ALU.add)
                            rq = w34.tile([DS, 128], F32, tag="rq", bufs=2,
                                          name="rq")
                            nc.vector.tensor_scalar(rq[:], zq[:], 0.0, None,
                                                    ALU.max)
                            eq = w34.tile([DS, 128], F32, tag="eq", bufs=2,
                                          name="eq")
                            nc.vector.tensor_tensor(eq[:], zq[:], rq[:],
                                                    ALU.subtract)
                            nc.scalar.activation(eq[:], eq[:], AF.Exp)
                            nc.vector.tensor_tensor(phiQT[:, i, :], rq[:], eq[:],
                                                    ALU.add)

                        # ---- C GEMM + phi_k_sum + pair allreduce ----
                        psc = [ps3.tile([DS, 512], F32, tag="c", bufs=2,
                                        name=f"psc{k}") for k in range(2)]
                        psk = ps3.tile([DS, 1], F32, tag="pks", bufs=1, name="psk")
                        for i in range(NT):
                            qn1t = w34.tile([128, DM], BF16, tag="qn1t", bufs=3,
                                            name="qn1t")
                            nc.sync.dma_start(qn1t[:],
                                              qn1_sp[i * 128:(i + 1) * 128, :])
                            for nb in range(2):
                                nc.tensor.matmul(psc[nb][:], phiK[:, i, :],
                                                 qn1t[:, nb * 512:(nb + 1) * 512],
                                                 start=(i == 0), stop=(i == NT - 1))
                            nc.tensor.matmul(psk[:], phiK[:, i, :], ones_b[:],
                                             start=(i == 0), stop=(i == NT - 1))
                        c_stage = w34.tile([DS, DM + 1], F32, tag="c_stage",
                                           name="c_stage")
                        for nb in range(2):
                            nc.vector.tensor_copy(
                                c_stage[:, nb * 512:(nb + 1) * 512], psc[nb][:])
                        nc.vector.tensor_copy(c_stage[:, DM:DM + 1], psk[:])
                        nc.gpsimd.dma_start(c_in[:], c_stage[:])
                        nc.gpsimd.collective_compute(
                            "AllReduce", ALU.add, replica_groups=GRP_PAIR,
                            ins=[c_in.opt()], outs=[c_out.opt()])

                        nc.gpsimd.dma_start(c_b[:], c_out[:, 0:DM])
                        nc.gpsimd.dma_start(pks_b[:], c_out[:, DM:DM + 1])

                    # ============ phase 5 ============
                    mT = transp.tile([128, ND, T], BF16, tag="tbig", name="mT")
                    with tc.tile_pool(name="w5", bufs=1) as w5, \
                         tc.tile_pool(name="ps5", bufs=1, space="PSUM") as ps5:
                        for i in range(NT):
                            pd = ps5.tile([128, 1], F32, tag="d1", bufs=2, name="pd")
                            nc.tensor.matmul(pd[:], phiQT[:, i, :], pks_b[:],
                                             start=True, stop=True)
                            ab = stats.tile([128, 1], F32, tag="ab", bufs=2,
                                            name="ab")
                            nc.scalar.activation(ab[:], pd[:], AF.Abs)
                            nc.vector.tensor_scalar(ab[:], ab[:], 1.0, None,
                                                    ALU.add)
                            nc.vector.reciprocal(rN[:, i:i + 1], ab[:])
                        for i in range(NT):
                            r = slice(i * 128, (i + 1) * 128)
                            qn1t = w5.tile([128, DM], F32, tag="qn1tf", bufs=2,
                                           name="qn1tf")
                            nc.gpsimd.dma_start(qn1t[:], qn1_sp[r, :])
                            mb = w5.tile([128, DM], BF16, tag="mb", bufs=2,
                                         name="mb")
                            for nb in range(2):
                                cs = slice(nb * 512, (nb + 1) * 512)
                                pa = ps5.tile([128, 512], F32, tag="mm512", bufs=4,
                                              name="pa")
                                nc.tensor.matmul(pa[:], phiQT[:, i, :], c_b[:, cs],
                                                 start=True, stop=True)
                                nc.vector.scalar_tensor_tensor(
                                    mb[:, cs], pa[:], rN[:, i:i + 1], qn1t[:, cs],
                                    ALU.mult, ALU.subtract)
                            pe_transpose_tile(ps5, mb, mT, r, ident_b)

                            baset = w5.tile([128, DM], F32, tag="baset", bufs=2,
                                            name="baset")
                            nc.sync.dma_start(baset[:], base_in[r, :])
                            qint = w5.tile([128, DM], F32, tag="qint", bufs=1,
                                           name="qint")
                            pms = [ps5.tile([128, 512], F32, tag="mm512", bufs=4,
                                            name=f"pm{k}") for k in range(2)]
                            for dj in range(ND):
                                for nb in range(2):
                                    cs = slice(nb * 512, (nb + 1) * 512)
                                    nc.tensor.matmul(pms[nb][:], mT[:, dj, r],
                                                     mprojb[:, dj, cs],
                                                     start=(dj == 0),
                                                     stop=(dj == ND - 1))
                            for nb in range(2):
                                cs = slice(nb * 512, (nb + 1) * 512)
                                nc.vector.scalar_tensor_tensor(
                                    qint[:, cs], pms[nb][:], dt_safe, baset[:, cs],
                                    ALU.mult, ALU.add)
                            qint_b = w5.tile([128, DM], BF16, tag="qint_b", bufs=2,
                                             name="qint_b")
                            nc.scalar.activation(qint_b[:], qint[:], AF.Copy)
                            nc.sync.dma_start(qint_sp[r, :], qint_b[:])
                            qn2b = w5.tile([128, DM], BF16, tag="qn2b", bufs=2,
                                           name="qn2b")
                            ln_tile(w5, stats, qint, n2g_bc, n2b_bc, eps_c[:],
                                    out_tile=qn2b)
                            pe_transpose_tile(ps5, qn2b, qn2T, r, ident_b)

            # ============ phases 6-7: FFN ============
            with tc.tile_pool(name="ffn", bufs=1) as ffn, \
                 tc.tile_pool(name="ws", bufs=1) as ws:
                H = ffn.tile([128, NIH, T], FP8, tag="H", name="H")
                for half in range(2):
                    ibase = half * NIH
                    sv = ws.tile([128, NIH, 4], BF16, tag="sv", name="sv")
                    with tc.tile_pool(name="psgu", bufs=1, space="PSUM") as psgu:
                        for mi in range(NIH):
                            g0 = half * 2048 + mi * 128
                            u0 = INNER + g0
                            wu_g = ws.tile([128, ND, 128], FP8, tag="wu_g", bufs=3,
                                           name="wu_g")
                            nc.gpsimd.dma_start(
                                wu_g[:],
                                wupT[:, g0:g0 + 128].rearrange(
                                    "(dj p) f -> p dj f", p=128))
                            wu_u = ws.tile([128, ND, 128], FP8, tag="wu_u", bufs=3,
                                           name="wu_u")
                            nc.gpsimd.dma_start(
                                wu_u[:],
                                wupT[:, u0:u0 + 128].rearrange(
                                    "(dj p) f -> p dj f", p=128))
                            htmp = ws.tile([128, T], FP8, tag="htmp", bufs=3,
                                           name="htmp")
                            for nb in range(2):
                                cs0 = nb * 1024
                                pg = psgu.tile([128, 1024], F32, tag="pg", bufs=2,
                                               name="pg")
                                pu = psgu.tile([128, 1024], F32, tag="pu", bufs=2,
                                               name="pu")
                                for hb in range(2):
                                    cs = slice(cs0 + hb * 512, cs0 + (hb + 1) * 512)
                                    ps_s = slice(hb * 512, (hb + 1) * 512)
                                    for dj in range(0, ND, 2):
                                        nc.tensor.matmul(pg[:, ps_s],
                                                         wu_g[:, dj:dj + 2, :],
                                                         qn2T[:, dj:dj + 2, cs],
                                                         start=(dj == 0),
                                                         stop=(dj == ND - 2),
                                                         perf_mode=DR)
                                    for dj in range(0, ND, 2):
                                        nc.tensor.matmul(pu[:, ps_s],
                                                         wu_u[:, dj:dj + 2, :],
                                                         qn2T[:, dj:dj + 2, cs],
                                                         start=(dj == 0),
                                                         stop=(dj == ND - 2),
                                                         perf_mode=DR)
                                gt = ws.tile([128, 1024], F32, tag="gt", bufs=2,
                                             name="gt")
                                nc.scalar.activation(gt[:], pg[:], AF.Silu)
                                nc.vector.tensor_tensor(htmp[:, cs0:cs0 + 1024],
                                                        gt[:], pu[:], ALU.mult)
                            # save raw boundary cols for the post-halo patch
                            nc.gpsimd.tensor_copy(sv[:, mi, 0:2], htmp[:, 0:2])
                            nc.gpsimd.tensor_copy(sv[:, mi, 2:4], htmp[:, T - 2:T])
                            # conv tokens 1..T-2 directly into H (edges patched)
                            w0 = dwk_sb[:, ibase + mi, 0:1]
                            wm = dwk_sb[:, ibase + mi, 1:2]
                            w2_ = dwk_sb[:, ibase + mi, 2:3]
                            hsl = H[:, mi, 1:T - 1]
                            nc.vector.tensor_scalar(hsl, htmp[:, 0:T - 2], w0,
                                                    None, ALU.mult)
                            nc.vector.scalar_tensor_tensor(hsl, htmp[:, 1:T - 1],
                                                           wm, hsl, ALU.mult,
                                                           ALU.add)
                            nc.vector.scalar_tensor_tensor(hsl, htmp[:, 2:T],
                                                           w2_, hsl, ALU.mult,
                                                           ALU.add)
                    # ---- halo exchange + boundary patch ----
                    nc.gpsimd.dma_start(halo_in[half][:, 0:NIH], sv[:, :, 0])
                    nc.gpsimd.dma_start(halo_in[half][:, NIH:2 * NIH], sv[:, :, 3])
                    nc.gpsimd.collective_compute(
                        "AllGather", ALU.bypass, replica_groups=GRP_PAIR,
                        ins=[halo_in[half].opt()], outs=[halo_out[half].opt()])
                    hg0 = ws.tile([128, 2 * NIH], BF16, tag="hg0", name="hg0")
                    nc.gpsimd.dma_start(hg0[:], halo_out[half][0])
                    hg1 = ws.tile([128, 2 * NIH], BF16, tag="hg1", name="hg1")
                    nc.gpsimd.dma_start(hg1[:], halo_out[half][1])
                    halL = ws.tile([128, NIH], BF16, tag="halL", name="halL")
                    nc.vector.tensor_scalar(halL[:], hg0[:, NIH:2 * NIH],
                                            lrsel_sb[:, 0:1], None, ALU.mult)
                    nc.vector.scalar_tensor_tensor(halL[:], hg1[:, NIH:2 * NIH],
                                                   lrsel_sb[:, 1:2], halL[:],
                                                   ALU.mult, ALU.add)
                    halR = ws.tile([128, NIH], BF16, tag="halR", name="halR")
                    nc.vector.tensor_scalar(halR[:], hg0[:, 0:NIH],
                                            lrsel_sb[:, 2:3], None, ALU.mult)
                    nc.vector.scalar_tensor_tensor(halR[:], hg1[:, 0:NIH],
                                                   lrsel_sb[:, 3:4], halR[:],
                                                   ALU.mult, ALU.add)
                    w0v = dwk_b[:, ibase:ibase + NIH, 0]
                    w1v = dwk_b[:, ibase:ibase + NIH, 1]
                    w2v = dwk_b[:, ibase:ibase + NIH, 2]
                    pt1 = ws.tile([128, NIH], BF16, tag="pt1", name="pt1")
                    pt2 = ws.tile([128, NIH], BF16, tag="pt2", name="pt2")
                    nc.vector.tensor_tensor(pt1[:], halL[:], w0v, ALU.mult)
                    nc.vector.tensor_tensor(pt2[:], sv[:, :, 0], w1v, ALU.mult)
                    nc.vector.tensor_tensor(pt1[:], pt1[:], pt2[:], ALU.add)
                    nc.vector.tensor_tensor(pt2[:], sv[:, :, 1], w2v, ALU.mult)
                    nc.vector.tensor_tensor(H[:, :, 0], pt1[:], pt2[:], ALU.add)
                    nc.vector.tensor_tensor(pt1[:], sv[:, :, 2], w0v, ALU.mult)
                    nc.vector.tensor_tensor(pt2[:], sv[:, :, 3], w1v, ALU.mult)
                    nc.vector.tensor_tensor(pt1[:], pt1[:], pt2[:], ALU.add)
                    nc.vector.tensor_tensor(pt2[:], halR[:], w2v, ALU.mult)
                    nc.vector.tensor_tensor(H[:, :, T - 1], pt1[:], pt2[:],
                                            ALU.add)
                    # ---- W_down GEMM (dm in 2 halves of 512) ----
                    with tc.tile_pool(name="pswd", bufs=1, space="PSUM") as pswd:
                        for dmq in range(2):
                            ns = slice(dmq * 512, (dmq + 1) * 512)
                            wd = ws.tile([128, NIH, 512], FP8, tag="wd", bufs=2,
                                         name="wd")
                            nc.gpsimd.dma_start(
                                wd[:],
                                wdownT[half * 2048:(half + 1) * 2048, ns]
                                .rearrange("(mi p) n -> p mi n", p=128))
                            for mt in list(range(1, NT - 1)) + [0, NT - 1]:
                                ms = slice(mt * 128, (mt + 1) * 128)
                                rr = slice(mt * 128, (mt + 1) * 128)
                                po = pswd.tile([128, 512], F32, tag="wdp", bufs=4,
                                               name="po")
                                for mi in range(0, NIH, 2):
                                    nc.tensor.matmul(po[:], H[:, mi:mi + 2, ms],
                                                     wd[:, mi:mi + 2, :],
                                                     start=(mi == 0),
                                                     stop=(mi == NIH - 2),
                                                     perf_mode=DR)
                                if half == 0:
                                    qiv = ws.tile([128, 512], BF16, tag="qiv",
                                                  bufs=2, name="qiv")
                                    nc.sync.dma_start(qiv[:], qint_sp[rr, ns])
                                    qif = ws.tile([128, 512], F32, tag="qif",
                                                  bufs=2, name="qif")
                                    nc.scalar.activation(qif[:], qiv[:], AF.Copy)
                                    a0 = ws.tile([128, 512], BF16, tag="a0",
                                                 bufs=2, name="a0")
                                    nc.vector.tensor_tensor(a0[:], po[:], qif[:],
                                                            ALU.add)
                                    nc.sync.dma_start(acc_sp[rr, ns], a0[:])
                                else:
                                    av = ws.tile([128, 512], BF16, tag="av",
                                                 bufs=2, name="av")
                                    nc.sync.dma_start(av[:], acc_sp[rr, ns])
                                    af = ws.tile([128, 512], F32, tag="af",
                                                 bufs=2, name="af")
                                    nc.scalar.activation(af[:], av[:], AF.Copy)
                                    ot = ws.tile([128, 512], F32, tag="ot",
                                                 bufs=2, name="ot")
                                    nc.vector.tensor_tensor(ot[:], po[:], af[:],
                                                            ALU.add)
                                    nc.sync.dma_start(out_ext[rr, ns], ot[:])

    nc.compile()
    return nc


def _build_cached(dt_safe, lam_safe, ident_norm):
    key = (round(float(dt_safe), 8), round(float(lam_safe), 8), bool(ident_norm))
    if key not in _cache:
        _cache[key] = build(float(dt_safe), float(lam_safe), bool(ident_norm))
    return _cache[key]


def kernel(**inputs):
    _install_ntff_shim()
    Q_in = np.ascontiguousarray(inputs["Q_in"], dtype=np.float32)
    X = np.ascontiguousarray(inputs["X"], dtype=np.float32)
    cos = np.ascontiguousarray(inputs["cos"], dtype=np.float32)
    sin = np.ascontiguousarray(inputs["sin"], dtype=np.float32)
    hyper_q_w = np.asarray(inputs["hyper_q_w"], dtype=np.float32)
    hyper_k_w = np.asarray(inputs["hyper_k_w"], dtype=np.float32)
    B_Q = np.asarray(inputs["B_Q"], dtype=np.float32)
    B_K = np.asarray(inputs["B_K"], dtype=np.float32)
    W_up = np.asarray(inputs["W_up"], dtype=np.float32)
    dw = np.asarray(inputs["dw_conv_w"], dtype=np.float32)
    W_down = np.asarray(inputs["W_down"], dtype=np.float32)
    m_proj_w = np.asarray(inputs["m_proj_w"], dtype=np.float32)
    m_proj_b = np.asarray(inputs["m_proj_b"], dtype=np.float32)
    n1g = np.asarray(inputs["norm1_g"], dtype=np.float32)
    n1b = np.asarray(inputs["norm1_b"], dtype=np.float32)
    n2g = np.asarray(inputs["norm2_g"], dtype=np.float32)
    n2b = np.asarray(inputs["norm2_b"], dtype=np.float32)
    dt = float(np.asarray(inputs["dt"]))
    lam = float(np.asarray(inputs["lam"]))

    dt_safe = float(np.log1p(np.exp(dt)))
    lam_safe = float(np.log1p(np.exp(lam)))
    ident_norm = bool(np.all(n1g == 1) and np.all(n1b == 0)
                      and np.all(n2g == 1) and np.all(n2b == 0)
                      and np.all(m_proj_b == 0))

    nc = _build_cached(dt_safe, lam_safe, ident_norm)

    # host-side sharding / layout prep
    hyperT = np.stack([hyper_q_w.T, hyper_k_w.T]).astype(ml_dtypes.bfloat16)
    wupT = np.ascontiguousarray(W_up.T).astype(ml_dtypes.float8_e4m3)
    wdownT = np.ascontiguousarray(W_down.T).astype(ml_dtypes.float8_e4m3)
    mprojT = np.ascontiguousarray(m_proj_w.T)
    nrm_bc = np.stack([
        np.broadcast_to(n1g, (128, DM)),
        np.broadcast_to(n1b, (128, DM)),
        np.broadcast_to(n2g, (128, DM)),
        np.broadcast_to(n2b, (128, DM)),
    ]).astype(np.float32)
    bias_q = np.stack([B_Q, -B_Q], axis=1).astype(np.float32)
    bias_k_bc = np.ascontiguousarray(np.broadcast_to(B_K, (128, DS)),
                                     dtype=np.float32)
    dwk = np.ascontiguousarray(
        dw[:, 0, :].reshape(2 * NIH, 128, 3).transpose(1, 0, 2),
        dtype=np.float32)
    base_full = ((1.0 - lam_safe) * Q_in + lam_safe * X
                 + (dt_safe * m_proj_b)[None, None, :]).astype(np.float32)

    in_maps = []
    for c in range(NCORES):
        b, h = c // 2, c % 2
        tok = slice(h * T, (h + 1) * T)
        rsl = slice(c * RS, (c + 1) * RS)
        bsel = np.zeros((128, B), np.float32)
        bsel[:, b] = 1.0
        lrsel = np.zeros((128, 4), np.float32)
        if h == 0:
            lrsel[:, 3] = 1.0
        else:
            lrsel[:, 0] = 1.0
        in_maps.append({
            "q_in": np.ascontiguousarray(Q_in[b, tok]),
            "base_in": np.ascontiguousarray(base_full[b, tok]),
            "cosn": np.ascontiguousarray(cos[tok]),
            "sinn": np.ascontiguousarray(sin[tok]),
            "hqk": np.ascontiguousarray(hyperT[:, :, rsl]),
            "wupT": wupT,
            "wdownT": wdownT,
            "mprojT": mprojT,
            "nrm_bc": nrm_bc,
            "bias_q": bias_q,
            "bias_k_bc": bias_k_bc,
            "dwk": dwk,
            "bsel": bsel,
            "lrsel": lrsel,
        })

    trace = bool(os.environ.get("BASS_KERNEL_TRACE"))
    res = run_bass_kernel_spmd(nc, in_maps, core_ids=list(range(NCORES)),
                               trace=trace,
                               tmpdir=os.environ.get("BASS_KERNEL_TMPDIR"))
    kernel._last = res

    out = np.empty((B, N, DM), np.float32)
    for c in range(NCORES):
        b, h = c // 2, c % 2
        out[b, h * T:(h + 1) * T] = res.results[c]["out"]
    return out
